# revision 1
# baseline (speedup 1.0000x reference)
"""Trainium2 Bass kernel for nn_AttentionDecoder (B=4, C=256, H=W=64).

Math (per batch b):
    q  = Wq @ x[b]  + bq          [32, N]   (as qT on device: [32, N] with o on partitions)
    k' = Wk @ xe[b] + bk + pos    [32, N]
    v  = Wv @ xe[b]               [256, N]  (bv folded into epilogue: sum(attn)=1)
    eT = k'^T-chunks: energy^T[m, n] = sum_o k'[o,m] q[o,n]
    pT = exp(eT)                  (no max-subtraction: |energy| < ~30, fp32-exp safe)
    out[c, n] = sum_m v[c, m] pT[m, n]      (PE: lhsT=vT chunk, rhs=pT chunk)
    s[n]      = sum_m pT[m, n]              (PE: lhsT=ones -> replicated rows)
    y = gamma * (out / s + bv) + x

Sharding: 8 cores = (batch, query-half). Each core: 2048 query rows, full m=4096.
"""

import numpy as np
import ml_dtypes
from contextlib import ExitStack

import concourse.bass as bass
import concourse.bacc as bacc
import concourse.tile as tile
import concourse.mybir as mybir
from concourse.bass import ds, ts

B, C, H, W = 4, 256, 64, 64
N = H * W          # 4096
C8 = 32
NH = N // 2        # 2048 query rows per core
NCORES = 8
NG = NH // 512     # 4 n-groups of 512 per core
F32 = mybir.dt.float32
BF16 = mybir.dt.bfloat16
AF = mybir.ActivationFunctionType
OP = mybir.AluOpType

LAST_EXEC_TIME_NS = None
_CACHE = {}


def build_attention(
    ctx,
    tc,
    y,
    ins,
    energy_mode="pack4",
    skip_s=False,
    lag=2,
    tree_levels=3,
    eps_split=False,
    wbufs=2,
    skip_recip=False,
    pv_first=False,
    finish_at=2,
):
    """y: [128, 2*NH] f32 dram AP.  ins: dict of dram APs (see kernel()).

    energy_mode:
      'pack4'  - 4x row-tiled K=32 matmuls (tile_position), k' strip-split
      'k32'    - plain K=32 matmuls at partitions 0:32 (no tile_position)
      'rep128' - k' replicated on all 4 strips, full K=128 matmul computes
                 4x energy; the 1/4 is folded into exp's scale (free affine)
    """
    nc = tc.nc
    y_v = y.rearrange("p (u n) -> p u n", u=2)
    xqf_d = ins["xqf"].rearrange("p (u n) -> p u n", u=2)
    xqb_d = ins["xqb"].rearrange("p (u n) -> p u n", u=2)
    xeb_d = ins["xeb"].rearrange("p (u m) -> p u m", u=2)

    singles = ctx.enter_context(tc.tile_pool(name="singles", bufs=1))

    # ---- resident SBUF tensors; DMA order matters: what productions need
    # first goes first, the residual input (only needed ~60us in) goes last.
    wq_sb = singles.tile([128, 2, 128], BF16, name="wq_sb")
    nc.sync.dma_start(wq_sb, ins["wq"].rearrange("p (u j) -> p u j", u=2))
    wk_sb = singles.tile([128, 2 * C8], BF16, name="wk_sb")
    nc.sync.dma_start(wk_sb, ins["wk"])
    wv_sb = singles.tile([128, 2 * C], BF16, name="wv_sb")
    nc.sync.dma_start(wv_sb, ins["wv"])
    bq_sb = singles.tile([128, 1], F32, name="bq_sb")
    nc.sync.dma_start(bq_sb, ins["bq"])
    kbias_sb = singles.tile([128, 8 * 128], BF16, name="kbias_sb")
    nc.sync.dma_start(kbias_sb, ins["kbias"])
    consts_sb = singles.tile([128, 3], F32, name="consts_sb")
    nc.sync.dma_start(consts_sb, ins["consts"])
    # xqb split per (c-chunk, n-group) so qT production starts on first chunk
    xqb_sb = singles.tile([128, 2, NH], BF16, name="xqb_sb")
    for g4 in range(NG):
        for u in range(2):
            nc.sync.dma_start(
                xqb_sb[:, u, ds(512 * g4, 512)], xqb_d[:, u, ds(512 * g4, 512)]
            )
    # xeb split into chunks so k/v production starts before the full 2MB lands
    xeb_sb = singles.tile([128, 2, N], BF16, name="xeb_sb")
    for u in range(2):
        for quarter in range(4):
            nc.sync.dma_start(
                xeb_sb[:, u, ds(1024 * quarter, 1024)],
                xeb_d[:, u, ds(1024 * quarter, 1024)],
            )
    xqf_sb = singles.tile([128, 2, NH], F32, name="xqf_sb")
    nc.sync.dma_start(xqf_sb, xqf_d)

    ones_sb = singles.tile([128, 128], BF16, name="ones_sb")
    nc.vector.memset(ones_sb, 1.0)

    # warm the Exp ACT table during the production phase (table load ~2.7us)
    act_warm = singles.tile([1, 1], F32, name="act_warm")
    nc.scalar.activation(out=act_warm, in_=bq_sb[0:1, :], func=AF.Exp)

    # qT4: qT replicated at 4 partition strips (strip i holds qT[o, :] at
    # partitions 32i+o) for the row-packed / replicated energyT matmuls.
    qT4_sb = singles.tile([128, NH], BF16, name="qT4_sb")
    # k4: k' distributed over strips: strip i = m in [1024i, 1024(i+1))
    assert energy_mode == "pack4", "col-tiled k production supports pack4 only"
    k4_sb = singles.tile([128, 8 * 128], BF16, name="k4_sb")
    vT_sb = singles.tile([128, 32 * C], BF16, name="vT_sb")

    # ---- productions ----
    with tc.tile_pool(name="prodpsum", bufs=2, space="PSUM") as prodpsum:
        # qT4[32i+o, n] = sum_c Wq[o, c] x[c, n] + bq[o]   (wq host-tiled x4)
        for g4 in range(NG):
            psq = prodpsum.tile([128, 512], F32, name="psq")
            for u in range(2):
                nc.tensor.matmul(
                    psq,
                    lhsT=wq_sb[:, u, :],
                    rhs=xqb_sb[:, u, ds(512 * g4, 512)],
                    start=(u == 0),
                    stop=(u == 1),
                )
            nc.vector.tensor_scalar_add(qT4_sb[:, ds(512 * g4, 512)], psq, bq_sb)
        # k'[o, m] = sum_c Wk[o, c] xe[c, m] + kbias[o, m], produced
        # col-tiled (4 concurrent partition strips) directly in strip layout
        psk4 = prodpsum.tile([128, 8 * 128], F32, name="psk4", bufs=1)
        for c8 in range(N // 512):
            i, j = c8 // 2, c8 % 2
            for u in range(2):
                nc.tensor.matmul(
                    psk4[ds(C8 * i, C8), ds(512 * j, 512)],
                    lhsT=wk_sb[:, ds(C8 * u, C8)],
                    rhs=xeb_sb[:, u, ds(512 * c8, 512)],
                    start=(u == 0),
                    stop=(u == 1),
                    tile_position=(0, C8 * i),
                )
        nc.vector.tensor_add(k4_sb, psk4, kbias_sb)
        # vT[m-chunk mc][mm, co] = sum_c xe[c, 128*mc+mm] Wv[co, c]
        def emit_vt_chunk(pool, mc):
            psv = pool.tile([128, C], F32, name="psv")
            for u in range(2):
                nc.tensor.matmul(
                    psv,
                    lhsT=xeb_sb[:, u, ds(128 * mc, 128)],
                    rhs=wv_sb[:, ds(C * u, C)],
                    start=(u == 0),
                    stop=(u == 1),
                )
            nc.vector.tensor_copy(vT_sb[:, ds(C * mc, C)], psv)

        for mc in range(32):
            emit_vt_chunk(prodpsum, mc)

    # ---- main loop: intra-group pipeline with 2-J-step lag.
    # energyT round J: 4 row-packed K=32 matmuls (strips i=0..3) -> 4 psum
    # banks; exp writes pT slots 4J..4J+3. PV of slots 4(J-2).. runs two
    # J-steps behind so PE has work while ACT drains exp. The s-reduction
    # (DVE tree 32->8 chunks + 8 partition-sum matmuls) and the epilogue of
    # group g overlap group g+1's pipeline fill.
    ppool = ctx.enter_context(tc.tile_pool(name="ppool", bufs=2))
    epool = ctx.enter_context(tc.tile_pool(name="epool", bufs=1, space="PSUM"))
    work = ctx.enter_context(tc.tile_pool(name="work", bufs=wbufs))
    mpsum = ctx.enter_context(tc.tile_pool(name="mpsum", bufs=1, space="PSUM"))
    # PSUM budget: epool 4 + pv0/pv1/s_ps 3 = 7 of 8 banks

    def slot_to_chunk(s):
        return 8 * (s % 4) + s // 4 if energy_mode == "pack4" else s

    LAG = lag

    def finish(p):
        """Tree-tail + s-matmuls + normalize + residual + store for group p."""
        gp = p["g"]
        s_ps = mpsum.tile([128, 512], F32, name="s_ps")
        if skip_s:
            nc.vector.memset(s_ps, 1.0)
        else:
            st_b = work.tile([128, 8, 512], BF16, name="st_b", bufs=1)
            nc.vector.tensor_add(
                st_b, p["pT"][:, 16:24, :], p["pT"][:, 24:32, :]
            )
            st_c = work.tile([128, 8, 512], BF16, name="st_c", bufs=1)
            nc.vector.tensor_add(st_c, p["st_a"], st_b)
            st3 = work.tile([128, 4, 512], BF16, name="st3", bufs=1)
            nc.vector.tensor_add(st3, st_c[:, 0:4, :], st_c[:, 4:8, :])
            for s8 in range(4):
                nc.tensor.matmul(
                    s_ps,
                    lhsT=ones_sb,
                    rhs=st3[:, s8, :],
                    start=(s8 == 0),
                    stop=(s8 == 3),
                )
        r_rep = work.tile([128, 512], F32, name="r_rep")
        if skip_recip:
            nc.vector.memset(r_rep, 1.0)
        else:
            nc.vector.reciprocal(r_rep, s_ps)
        for u, ou in enumerate([p["o0"], p["o1"]]):
            t = work.tile([128, 512], F32, name="t")
            nc.vector.scalar_tensor_tensor(
                out=t,
                in0=ou,
                scalar=consts_sb[:, 0:1],
                in1=r_rep,
                op0=OP.mult,
                op1=OP.mult,
            )
            yt = work.tile([128, 512], F32, name="yt")
            nc.vector.scalar_tensor_tensor(
                out=yt,
                in0=t,
                scalar=consts_sb[:, u + 1 : u + 2],
                in1=xqf_sb[:, u, ds(512 * gp, 512)],
                op0=OP.add,
                op1=OP.add,
            )
            nc.sync.dma_start(y_v[:, u, ds(512 * gp, 512)], yt)

    pending = None
    for g in range(NG):
        pT = ppool.tile([128, 32, 512], BF16, name="pT")
        pv0 = mpsum.tile([128, 512], F32, name="pv0")
        pv1 = mpsum.tile([128, 512], F32, name="pv1")
        st_a = None
        def emit_e(J):
                if eps_split:
                    e_lo = epool.tile([128, 2, 512], F32, name="e_lo")
                    e_hi = epool.tile([128, 2, 512], F32, name="e_hi")
                    halves = [e_lo, e_hi]
                    e_ps = None
                else:
                    e_ps = epool.tile([128, 4, 512], F32, name="e_ps")
                for i in range(4):
                    e_out = (
                        halves[i // 2][:, i % 2, :] if eps_split else e_ps[:, i, :]
                    )
                    nc.tensor.matmul(
                        e_out,
                        lhsT=k4_sb[ds(C8 * i, C8), ds(128 * J, 128)],
                        rhs=qT4_sb[ds(C8 * i, C8), ds(512 * g, 512)],
                        start=True,
                        stop=True,
                        tile_position=(C8 * i, 0),
                    )
                escale = 0.25 if energy_mode == "rep128" else 1.0
                if eps_split:
                    for h in range(2):
                        nc.scalar.activation(
                            out=pT[:, ds(4 * J + 2 * h, 2), :],
                            in_=halves[h],
                            func=AF.Exp,
                            scale=escale,
                        )
                else:
                    nc.scalar.activation(
                        out=pT[:, ds(4 * J, 4), :],
                        in_=e_ps,
                        func=AF.Exp,
                        scale=escale,
                    )

        def emit_pv(J):
                for j in range(4):
                    s = 4 * (J - LAG) + j
                    mc = slot_to_chunk(s)
                    st = s == 0
                    sp = s == 31
                    nc.tensor.matmul(
                        pv0,
                        lhsT=vT_sb[:, ds(C * mc, 128)],
                        rhs=pT[:, s, :],
                        start=st,
                        stop=sp,
                    )
                    nc.tensor.matmul(
                        pv1,
                        lhsT=vT_sb[:, ds(C * mc + 128, 128)],
                        rhs=pT[:, s, :],
                        start=st,
                        stop=sp,
                    )

        for J in range(8 + LAG):
            if pv_first:
                if J >= LAG:
                    emit_pv(J)
                if J < 8:
                    emit_e(J)
            else:
                if J < 8:
                    emit_e(J)
                if J >= LAG:
                    emit_pv(J)
            if J == 4 and not skip_s:
                # first half of the s slot-tree: slots 0..15 are ready
                st_a = work.tile([128, 8, 512], BF16, name="st_a")
                nc.vector.tensor_add(st_a, pT[:, 0:8, :], pT[:, 8:16, :])
            if J == finish_at and pending is not None:
                finish(pending)
                pending = None
        # evacuate PV psum to SBUF right away so the psum banks free for the
        # next group's PV; the tree-tail + s-matmuls + normalize/epilogue are
        # deferred into the next group's J-loop (see finish())
        o0 = work.tile([128, 512], F32, name="o0")
        nc.vector.tensor_copy(o0, pv0)
        o1 = work.tile([128, 512], F32, name="o1")
        nc.vector.tensor_copy(o1, pv1)
        pending = dict(g=g, pT=pT, o0=o0, o1=o1, st_a=st_a)
    finish(pending)


INPUT_SPECS = [
    ("xqf", [128, 2 * NH], F32),
    ("xqb", [128, 2 * NH], BF16),
    ("xeb", [128, 2 * N], BF16),
    ("wq", [128, 2 * 128], BF16),
    ("wk", [128, 2 * C8], BF16),
    ("wv", [128, 2 * C], BF16),
    ("kbias", [128, 8 * 128], BF16),
    ("bq", [128, 1], F32),
    ("consts", [128, 3], F32),
]


def _get_program(loop_iters=None, **opts):
    """loop_iters=None: plain program. loop_iters=k: whole kernel wrapped in a
    device-side For_i loop (for HW timing: slope between two loop counts)."""
    key = ("nc", loop_iters, tuple(sorted(opts.items())))
    if key not in _CACHE:
        nc = bacc.Bacc("TRN2", debug=False, num_devices=NCORES)
        with tile.TileContext(nc) as tc:
            with ExitStack() as ctx:
                ins = {
                    name: nc.dram_tensor(name, shape, dt, kind="ExternalInput").ap()
                    for name, shape, dt in INPUT_SPECS
                }
                y = nc.dram_tensor("y", [128, 2 * NH], F32, kind="ExternalOutput").ap()
                if loop_iters is None:
                    build_attention(ctx, tc, y, ins, **opts)
                else:
                    # hint_engines: body >256 insts/engine, so the back-edge
                    # would otherwise pay an IRAM refetch (~3-4us) per iter
                    with tc.For_i(
                        0,
                        loop_iters,
                        1,
                        hint_engines=(
                            mybir.EngineType.PE,
                            mybir.EngineType.Activation,
                            mybir.EngineType.DVE,
                        ),
                    ):
                        with ExitStack() as inner:
                            build_attention(inner, tc, y, ins, **opts)
        nc.compile()
        _CACHE[key] = nc
    return _CACHE[key]


class _Runner:
    """Executes the compiled Bass program on 8 cores via PJRT/axon.

    Mirrors bass2jax.run_bass_via_pjrt's multi-core path, but keeps the
    jitted callable so repeated executions don't re-lower, and supports
    chaining `iters` NEFF executions inside one program (each iteration's
    outputs feed the next iteration's output buffers, creating a data
    dependency) so per-execution device time can be measured without
    host dispatch overhead.
    """

    def __init__(self, nc):
        import jax
        from jax.experimental.shard_map import shard_map
        from jax.sharding import Mesh, PartitionSpec
        from concourse import bass2jax

        bass2jax.install_neuronx_cc_hook()
        self.nc = nc
        self.jax = jax
        in_names, out_names, out_avals, zero_outs = [], [], [], []
        partition_name = (
            nc.partition_id_tensor.name if nc.partition_id_tensor else None
        )
        for alloc in nc.m.functions[0].allocations:
            if not isinstance(alloc, mybir.MemoryLocationSet):
                continue
            name = alloc.memorylocations[0].name
            if alloc.kind == "ExternalInput":
                if name != partition_name:
                    in_names.append(name)
            elif alloc.kind == "ExternalOutput":
                out_names.append(name)
                shape = tuple(alloc.tensor_shape)
                dtype = mybir.dt.np(alloc.dtype)
                out_avals.append(jax.core.ShapedArray(shape, dtype))
                zero_outs.append(np.zeros(shape, dtype))
        self.n_params = len(in_names)
        self.n_outs = len(out_avals)
        self.out_names = out_names
        self.out_avals = out_avals
        self.zero_outs = zero_outs
        all_in_names = list(in_names) + list(out_names)
        if partition_name is not None:
            all_in_names.append(partition_name)
        self.in_names = in_names
        self.partition_name = partition_name

        devices = jax.devices()[:NCORES]
        assert len(devices) == NCORES
        mesh = Mesh(np.asarray(devices), ("core",))
        donate = tuple(range(self.n_params, self.n_params + self.n_outs))
        out_avals_t = tuple(out_avals)
        all_in_names_t = tuple(all_in_names)
        out_names_t = tuple(out_names)

        self.mesh = mesh
        self.pspec = PartitionSpec("core")

        def make(donated):
            def _body(*args):
                operands = list(args)
                if partition_name is not None:
                    operands.append(bass2jax.partition_id_tensor())
                outs = bass2jax._bass_exec_p.bind(
                    *operands,
                    out_avals=out_avals_t,
                    in_names=all_in_names_t,
                    out_names=out_names_t,
                    lowering_input_output_aliases=(),
                    sim_require_finite=True,
                    sim_require_nnan=True,
                    nc=nc,
                )
                return tuple(outs)

            in_specs = (PartitionSpec("core"),) * (self.n_params + self.n_outs)
            out_specs = (PartitionSpec("core"),) * self.n_outs
            return jax.jit(
                shard_map(
                    _body,
                    mesh=mesh,
                    in_specs=in_specs,
                    out_specs=out_specs,
                    check_rep=False,
                ),
                donate_argnums=donate if donated else (),
                keep_unused=True,
            )

        self._make = make
        self._fns = {}

    def _fn(self, donated):
        if donated not in self._fns:
            self._fns[donated] = self._make(donated)
        return self._fns[donated]

    def _concat_args(self, in_maps):
        concat_in = [
            np.concatenate([np.asarray(m[name]) for m in in_maps], axis=0)
            for name in self.in_names
        ]
        concat_zeros = [
            np.zeros((NCORES * z.shape[0], *z.shape[1:]), z.dtype)
            for z in self.zero_outs
        ]
        return concat_in + concat_zeros

    def device_args(self, in_maps):
        """Pre-place sharded args on the 8 devices (for re-execution timing)."""
        jax = self.jax
        from jax.sharding import NamedSharding

        sharding = NamedSharding(self.mesh, self.pspec)
        return [jax.device_put(a, sharding) for a in self._concat_args(in_maps)]

    def execute(self, dev_args):
        """Run on pre-placed device args without donation; returns jax arrays."""
        return self._fn(False)(*dev_args)

    def run(self, in_maps):
        out_arrs = self._fn(True)(*self._concat_args(in_maps))
        out_arrs = [np.asarray(a) for a in out_arrs]
        return [
            {
                name: out_arrs[i].reshape(NCORES, *self.out_avals[i].shape)[c]
                for i, name in enumerate(self.out_names)
            }
            for c in range(NCORES)
        ]


def get_runner():
    if "runner" not in _CACHE:
        _CACHE["runner"] = _Runner(_get_program())
    return _CACHE["runner"]


def get_loop_runner(loop_iters, **opts):
    key = ("runner", loop_iters, tuple(sorted(opts.items())))
    if key not in _CACHE:
        _CACHE[key] = _Runner(_get_program(loop_iters, **opts))
    return _CACHE[key]


def measure_hw_ns(in_maps, k_lo=1, k_hi=129, reps=6, **opts):
    """Per-iteration device time via two For_i loop-count variants."""
    import time as _time
    import jax as _jax

    def bench(runner):
        dev = runner.device_args(in_maps)
        for _ in range(2):
            _jax.block_until_ready(runner.execute(dev))
        best = float("inf")
        for _ in range(reps):
            t0 = _time.perf_counter()
            _jax.block_until_ready(runner.execute(dev))
            best = min(best, _time.perf_counter() - t0)
        return best

    t_lo = bench(get_loop_runner(k_lo, **opts))
    t_hi = bench(get_loop_runner(k_hi, **opts))
    return (t_hi - t_lo) / (k_hi - k_lo) * 1e9, t_lo, t_hi


def get_trivial_runner():
    """Near-empty NEFF (one tiny DMA in->out) to measure dispatch overhead."""
    if "trivial" not in _CACHE:
        nc = bacc.Bacc("TRN2", debug=False, num_devices=NCORES)
        with tile.TileContext(nc) as tc:
            with ExitStack() as ctx:
                tin = nc.dram_tensor("tin", [128, 8], F32, kind="ExternalInput").ap()
                tout = nc.dram_tensor(
                    "tout", [128, 8], F32, kind="ExternalOutput"
                ).ap()
                pool = ctx.enter_context(tc.tile_pool(name="tpool", bufs=1))
                tt = pool.tile([128, 8], F32, name="tt")
                nc.sync.dma_start(tt, tin)
                nc.sync.dma_start(tout, tt)
        nc.compile()
        _CACHE["trivial"] = _Runner(nc)
    return _CACHE["trivial"]


def _to2(a):
    """[256, X] -> [128, 2X] with out[p, u*X + j] = a[128u + p, j]."""
    x = np.asarray(a)
    return np.ascontiguousarray(
        x.reshape(2, 128, x.shape[1]).transpose(1, 0, 2).reshape(128, -1)
    )


def _bf(a):
    return np.ascontiguousarray(np.asarray(a, dtype=ml_dtypes.bfloat16))


def _f32(a):
    return np.ascontiguousarray(np.asarray(a, dtype=np.float32))


def kernel(x, x_encoder, Wq, bq, Wk, bk, Wv, bv, h_pos, w_pos, gamma):
    global LAST_EXEC_TIME_NS
    in_maps = make_in_maps(
        x, x_encoder, Wq, bq, Wk, bk, Wv, bv, h_pos, w_pos, gamma
    )
    runner = get_runner()
    results = runner.run(in_maps)

    out = np.empty((B, C, N), np.float32)
    for core in range(NCORES):
        b, half = divmod(core, 2)
        yc = results[core]["y"]  # [128, 2*NH]
        out[b][:, half * NH : (half + 1) * NH] = (
            yc.reshape(128, 2, NH).transpose(1, 0, 2).reshape(C, NH)
        )
    return out.reshape(B, C, H, W)


def make_in_maps(x, x_encoder, Wq, bq, Wk, bk, Wv, bv, h_pos, w_pos, gamma):
    """Host-side input prep shared by kernel() and timing harnesses."""
    x = _f32(x)
    x_encoder = _f32(x_encoder)
    Wq, bq, Wk, bk, Wv, bv = map(_f32, (Wq, bq, Wk, bk, Wv, bv))
    h_pos, w_pos, gamma = map(_f32, (h_pos, w_pos, gamma))
    xf = x.reshape(B, C, N)
    xe = x_encoder.reshape(B, C, N)
    pos = (h_pos + w_pos).reshape(C8, N)
    kb = bk[:, None] + pos  # [32, 4096]
    # strip layout: kbias4[32i+o, j] = kb[o, 1024i + j]
    kbias = _bf(kb.reshape(C8, 4, 8 * 128).transpose(1, 0, 2).reshape(128, 8 * 128))
    wqT = Wq.T  # [256, 32]
    wq_h = _bf(
        np.concatenate(
            [np.tile(wqT[128 * u : 128 * (u + 1)], (1, 4)) for u in range(2)],
            axis=1,
        )
    )  # [128, 256]: wq_h[p, 128u + 32i + o] = Wq[o, 128u + p]
    wk_h = _bf(_to2(Wk.T))
    wv_h = _bf(_to2(Wv.T))
    bq_h = _f32(np.tile(bq, 4)[:, None])  # [128, 1]
    g = float(gamma.reshape(-1)[0])
    consts = np.empty((128, 3), np.float32)
    consts[:, 0] = g
    consts[:, 1] = g * bv[0:128]
    consts[:, 2] = g * bv[128:256]
    in_maps = []
    for core in range(NCORES):
        b, half = divmod(core, 2)
        xq = _to2(xf[b][:, half * NH : (half + 1) * NH])
        in_maps.append(
            {
                "xqf": _f32(xq),
                "xqb": _bf(xq),
                "xeb": _bf(_to2(xe[b])),
                "wq": wq_h,
                "wk": wk_h,
                "wv": wv_h,
                "kbias": kbias,
                "bq": bq_h,
                "consts": consts,
            }
        )
    return in_maps


if __name__ == "__main__":
    import reference

    inputs = {k: np.asarray(v) for k, v in reference.setup_inputs().items()}
    got = kernel(**inputs)
    print("kernel ran; output shape", got.shape, "exec_ns", LAST_EXEC_TIME_NS)



# revision 7
# speedup vs baseline: 1.0040x; 1.0040x over previous
"""Trainium2 Bass kernel for nn_AttentionDecoder (B=4, C=256, H=W=64).

Math (per batch b):
    q  = Wq @ x[b]  + bq          [32, N]   (as qT on device: [32, N] with o on partitions)
    k' = Wk @ xe[b] + bk + pos    [32, N]
    v  = Wv @ xe[b]               [256, N]  (bv folded into epilogue: sum(attn)=1)
    eT = k'^T-chunks: energy^T[m, n] = sum_o k'[o,m] q[o,n]
    pT = exp(eT)                  (no max-subtraction: |energy| < ~30, fp32-exp safe)
    out[c, n] = sum_m v[c, m] pT[m, n]      (PE: lhsT=vT chunk, rhs=pT chunk)
    s[n]      = sum_m pT[m, n]              (PE: lhsT=ones -> replicated rows)
    y = gamma * (out / s + bv) + x

Sharding: 8 cores = (batch, query-half). Each core: 2048 query rows, full m=4096.
"""

import numpy as np
import ml_dtypes
from contextlib import ExitStack

import concourse.bass as bass
import concourse.bass_isa as bass_isa
import concourse.bacc as bacc
import concourse.tile as tile
import concourse.mybir as mybir
from concourse.bass import ds, ts

B, C, H, W = 4, 256, 64, 64
N = H * W          # 4096
C8 = 32
NH = N // 2        # 2048 query rows per core
NCORES = 8
NG = NH // 512     # 4 n-groups of 512 per core
F32 = mybir.dt.float32
BF16 = mybir.dt.bfloat16
AF = mybir.ActivationFunctionType
OP = mybir.AluOpType

LAST_EXEC_TIME_NS = None
_CACHE = {}


def build_attention_v2(
    ctx,
    tc,
    y,
    ins,
    lag=2,
    vt_act_share=2,
    finish_at=0,
):
    """v2: stall-free PE schedule.

    - energy psum double-buffered as [128,2,512] pairs (4 banks), exp in
      2-slot instructions so round J+1's energy never waits a 4-slot drain
    - pv psum [128,2,512] double-buffered across groups (4 banks); the
      epilogue STTs read pv psum directly (no o0/o1 evacuation)
    - s-reduction: slot-tree split across Pool (st_a) and DVE, reduced to
      [128,512] with only the last-8-slot fold on the tail critical path;
      final cross-partition sum via gpsimd partition_all_reduce (no PE
      s-matmuls, no s psum bank)
    - input DMA split across SP and Pool queues in consumption order;
      vT psum evacuation split DVE/ACT; q bias-add on ACT (Copy+bias)
    """
    nc = tc.nc
    y_v = y.rearrange("p (u n) -> p u n", u=2)
    xqf_d = ins["xqf"].rearrange("p (u n) -> p u n", u=2)
    xqb_d = ins["xqb"].rearrange("p (u n) -> p u n", u=2)
    xeb_d = ins["xeb"].rearrange("p (u m) -> p u m", u=2)

    singles = ctx.enter_context(tc.tile_pool(name="singles", bufs=1))

    # ---- resident SBUF tensors ----
    wq_sb = singles.tile([128, 2, 128], BF16, name="wq_sb")
    bq_sb = singles.tile([128, 1], F32, name="bq_sb")
    wk_sb = singles.tile([128, 2 * C8], BF16, name="wk_sb")
    kbias_sb = singles.tile([128, 8 * 128], BF16, name="kbias_sb")
    wv_sb = singles.tile([128, 2 * C], BF16, name="wv_sb")
    consts_sb = singles.tile([128, 3], F32, name="consts_sb")
    xqb_sb = singles.tile([128, 2, NH], BF16, name="xqb_sb")
    xeb_sb = singles.tile([128, 2, N], BF16, name="xeb_sb")
    xqf_sb = singles.tile([128, 2, NH], F32, name="xqf_sb")
    qT4_sb = singles.tile([128, NH], BF16, name="qT4_sb")
    k4_sb = singles.tile([128, 8 * 128], BF16, name="k4_sb")
    vT_sb = singles.tile([128, 32 * C], BF16, name="vT_sb")

    # SP DMA queue, ordered to match PE consumption: q prod, k prod, vT prod
    nc.sync.dma_start(wq_sb, ins["wq"].rearrange("p (u j) -> p u j", u=2))
    nc.sync.dma_start(bq_sb, ins["bq"])
    for u in range(2):
        nc.sync.dma_start(xqb_sb[:, u, ds(0, 512)], xqb_d[:, u, ds(0, 512)])
    nc.sync.dma_start(wk_sb, ins["wk"])
    for quarter in range(4):
        for u in range(2):
            nc.sync.dma_start(
                xeb_sb[:, u, ds(1024 * quarter, 1024)],
                xeb_d[:, u, ds(1024 * quarter, 1024)],
            )
        if quarter == 0:
            nc.sync.dma_start(kbias_sb, ins["kbias"])
    # Pool DMA queue: weights/residual not needed until later; Pool engine
    # is idle until the first st_a (~20us in)
    nc.gpsimd.dma_start(wv_sb, ins["wv"])
    nc.gpsimd.dma_start(consts_sb, ins["consts"])
    for g4 in range(1, NG):
        for u in range(2):
            nc.gpsimd.dma_start(
                xqb_sb[:, u, ds(512 * g4, 512)], xqb_d[:, u, ds(512 * g4, 512)]
            )
    nc.gpsimd.dma_start(xqf_sb, xqf_d)

    # warm the Exp ACT table during the production phase (table load ~2.7us)
    act_warm = singles.tile([1, 1], F32, name="act_warm")
    nc.scalar.activation(out=act_warm, in_=bq_sb[0:1, :], func=AF.Exp)

    # ---- productions ----
    vT32 = vT_sb.rearrange("p (d c) -> p d c", c=C)
    with tc.tile_pool(name="psqp", bufs=2, space="PSUM") as psqp, tc.tile_pool(
        name="pskp", bufs=1, space="PSUM"
    ) as pskp, tc.tile_pool(name="psvp", bufs=3, space="PSUM") as psvp:

        def emit_q(g4):
            psq = psqp.tile([128, 512], F32, name="psq")
            for u in range(2):
                nc.tensor.matmul(
                    psq,
                    lhsT=wq_sb[:, u, :],
                    rhs=xqb_sb[:, u, ds(512 * g4, 512)],
                    start=(u == 0),
                    stop=(u == 1),
                )
            nc.scalar.activation(
                out=qT4_sb[:, ds(512 * g4, 512)], in_=psq, func=AF.Identity, bias=bq_sb
            )

        psk4 = pskp.tile([128, 8 * 128], F32, name="psk4")

        def emit_k(c8):
            i, j = c8 // 2, c8 % 2
            for u in range(2):
                nc.tensor.matmul(
                    psk4[ds(C8 * i, C8), ds(512 * j, 512)],
                    lhsT=wk_sb[:, ds(C8 * u, C8)],
                    rhs=xeb_sb[:, u, ds(512 * c8, 512)],
                    start=(u == 0),
                    stop=(u == 1),
                    tile_position=(0, C8 * i),
                )

        def emit_vt(t):
            # pair of m-positions (2t, 2t+1); evac alternates DVE/ACT
            psv = psvp.tile([128, 2, C], F32, name="psv")
            for w in range(2):
                for u in range(2):
                    nc.tensor.matmul(
                        psv[:, w, :],
                        lhsT=xeb_sb[:, u, ds(128 * (2 * t + w), 128)],
                        rhs=wv_sb[:, ds(C * u, C)],
                        start=(u == 0),
                        stop=(u == 1),
                    )
            dest = vT32[:, ds(2 * t, 2), :]
            if vt_act_share and t % vt_act_share == 0:
                nc.scalar.copy(dest, psv)
            else:
                nc.vector.tensor_copy(dest, psv)

        for g4 in range(NG):
            emit_q(g4)
        emit_k(0)
        emit_k(1)
        emit_vt(0)
        emit_vt(1)
        emit_k(2)
        emit_k(3)
        emit_vt(2)
        emit_vt(3)
        emit_vt(4)
        emit_vt(5)
        emit_k(4)
        emit_k(5)
        emit_vt(6)
        emit_vt(7)
        emit_k(6)
        emit_k(7)
        nc.vector.tensor_add(k4_sb, psk4, kbias_sb)
        for t in range(8, 16):
            emit_vt(t)

    # ---- main loop ----
    ppool = ctx.enter_context(tc.tile_pool(name="ppool", bufs=2))
    epool = ctx.enter_context(tc.tile_pool(name="epool", bufs=2, space="PSUM"))
    mpsum = ctx.enter_context(tc.tile_pool(name="mpsum", bufs=2, space="PSUM"))
    work = ctx.enter_context(tc.tile_pool(name="work", bufs=2))
    # PSUM budget: epool 2 tiles x 2 banks + mpsum 2 tiles x 2 banks = 8

    def slot_to_chunk(s):
        return 8 * (s % 4) + s // 4

    LAG = lag

    def finish(p):
        """Tail-8-slot fold + s + normalize + residual + store for group p
        (runs during the next group's early rounds; reads pv psum)."""
        gp = p["g"]
        b2 = work.tile([128, 4, 512], BF16, name="b2")
        nc.vector.tensor_add(b2, p["pT"][:, 24:28, :], p["pT"][:, 28:32, :])
        b2a = work.tile([128, 2, 512], BF16, name="b2a")
        nc.vector.tensor_add(b2a, b2[:, 0:2, :], b2[:, 2:4, :])
        b2b = work.tile([128, 512], BF16, name="b2b")
        nc.vector.tensor_add(b2b, b2a[:, 0, :], b2a[:, 1, :])
        st = work.tile([128, 512], BF16, name="st")
        nc.vector.tensor_add(st, p["c3"], b2b)
        s_rep = work.tile([128, 512], F32, name="s_rep")
        nc.gpsimd.partition_all_reduce(
            s_rep, st, channels=128, reduce_op=bass_isa.ReduceOp.add
        )
        r_rep = work.tile([128, 512], F32, name="r_rep")
        nc.vector.reciprocal(r_rep, s_rep)
        for u in range(2):
            t = work.tile([128, 512], F32, name="t")
            nc.vector.scalar_tensor_tensor(
                out=t,
                in0=p["pv"][:, u, :],
                scalar=consts_sb[:, 0:1],
                in1=r_rep,
                op0=OP.mult,
                op1=OP.mult,
            )
            yt = work.tile([128, 512], F32, name="yt")
            nc.vector.scalar_tensor_tensor(
                out=yt,
                in0=t,
                scalar=consts_sb[:, u + 1 : u + 2],
                in1=xqf_sb[:, u, ds(512 * gp, 512)],
                op0=OP.add,
                op1=OP.add,
            )
            nc.sync.dma_start(y_v[:, u, ds(512 * gp, 512)], yt)

    pending = None
    for g in range(NG):
        pT = ppool.tile([128, 32, 512], BF16, name="pT")
        pv = mpsum.tile([128, 2, 512], F32, name="pv")
        cur = {"g": g, "pT": pT, "pv": pv}
        for J in range(8 + LAG):
            if J < 8:
                for h in range(2):
                    e2 = epool.tile([128, 2, 512], F32, name="e2")
                    for k in range(2):
                        i = 2 * h + k
                        nc.tensor.matmul(
                            e2[:, k, :],
                            lhsT=k4_sb[ds(C8 * i, C8), ds(128 * J, 128)],
                            rhs=qT4_sb[ds(C8 * i, C8), ds(512 * g, 512)],
                            start=True,
                            stop=True,
                            tile_position=(C8 * i, 0),
                        )
                    nc.scalar.activation(
                        out=pT[:, ds(4 * J + 2 * h, 2), :], in_=e2, func=AF.Exp
                    )
            if J >= LAG and J - LAG < 8:
                for j in range(4):
                    s = 4 * (J - LAG) + j
                    mc = slot_to_chunk(s)
                    st = s == 0
                    sp = s == 31
                    for u in range(2):
                        nc.tensor.matmul(
                            pv[:, u, :],
                            lhsT=vT_sb[:, ds(C * mc + 128 * u, 128)],
                            rhs=pT[:, s, :],
                            start=st,
                            stop=sp,
                        )
            if J == 4:
                # Pool is otherwise idle: big first-level fold of slots 0:16
                cur["st_a"] = work.tile([128, 8, 512], BF16, name="st_a")
                nc.gpsimd.tensor_add(
                    cur["st_a"], pT[:, 0:8, :], pT[:, 8:16, :]
                )
            if J == 5:
                cur["b1"] = work.tile([128, 4, 512], BF16, name="b1")
                nc.vector.tensor_add(cur["b1"], pT[:, 16:20, :], pT[:, 20:24, :])
            if J == 6:
                cur["a2"] = work.tile([128, 4, 512], BF16, name="a2")
                nc.vector.tensor_add(
                    cur["a2"], cur["st_a"][:, 0:4, :], cur["st_a"][:, 4:8, :]
                )
                cur["cc"] = work.tile([128, 4, 512], BF16, name="cc")
                nc.vector.tensor_add(cur["cc"], cur["a2"], cur["b1"])
            if J == 7:
                cur["c2"] = work.tile([128, 2, 512], BF16, name="c2")
                nc.vector.tensor_add(
                    cur["c2"], cur["cc"][:, 0:2, :], cur["cc"][:, 2:4, :]
                )
                cur["c3"] = work.tile([128, 512], BF16, name="c3")
                nc.vector.tensor_add(cur["c3"], cur["c2"][:, 0, :], cur["c2"][:, 1, :])
            if J == finish_at and pending is not None:
                finish(pending)
                pending = None
        pending = cur
    finish(pending)


def build_attention(
    ctx,
    tc,
    y,
    ins,
    energy_mode="pack4",
    skip_s=False,
    lag=2,
    tree_levels=3,
    eps_split=False,
    wbufs=2,
    skip_recip=False,
    pv_first=False,
    finish_at=2,
):
    """y: [128, 2*NH] f32 dram AP.  ins: dict of dram APs (see kernel()).

    energy_mode:
      'pack4'  - 4x row-tiled K=32 matmuls (tile_position), k' strip-split
      'k32'    - plain K=32 matmuls at partitions 0:32 (no tile_position)
      'rep128' - k' replicated on all 4 strips, full K=128 matmul computes
                 4x energy; the 1/4 is folded into exp's scale (free affine)
    """
    nc = tc.nc
    y_v = y.rearrange("p (u n) -> p u n", u=2)
    xqf_d = ins["xqf"].rearrange("p (u n) -> p u n", u=2)
    xqb_d = ins["xqb"].rearrange("p (u n) -> p u n", u=2)
    xeb_d = ins["xeb"].rearrange("p (u m) -> p u m", u=2)

    singles = ctx.enter_context(tc.tile_pool(name="singles", bufs=1))

    # ---- resident SBUF tensors; DMA order matters: what productions need
    # first goes first, the residual input (only needed ~60us in) goes last.
    wq_sb = singles.tile([128, 2, 128], BF16, name="wq_sb")
    nc.sync.dma_start(wq_sb, ins["wq"].rearrange("p (u j) -> p u j", u=2))
    wk_sb = singles.tile([128, 2 * C8], BF16, name="wk_sb")
    nc.sync.dma_start(wk_sb, ins["wk"])
    wv_sb = singles.tile([128, 2 * C], BF16, name="wv_sb")
    nc.sync.dma_start(wv_sb, ins["wv"])
    bq_sb = singles.tile([128, 1], F32, name="bq_sb")
    nc.sync.dma_start(bq_sb, ins["bq"])
    kbias_sb = singles.tile([128, 8 * 128], BF16, name="kbias_sb")
    nc.sync.dma_start(kbias_sb, ins["kbias"])
    consts_sb = singles.tile([128, 3], F32, name="consts_sb")
    nc.sync.dma_start(consts_sb, ins["consts"])
    # xqb split per (c-chunk, n-group) so qT production starts on first chunk
    xqb_sb = singles.tile([128, 2, NH], BF16, name="xqb_sb")
    for g4 in range(NG):
        for u in range(2):
            nc.sync.dma_start(
                xqb_sb[:, u, ds(512 * g4, 512)], xqb_d[:, u, ds(512 * g4, 512)]
            )
    # xeb split into chunks so k/v production starts before the full 2MB lands
    xeb_sb = singles.tile([128, 2, N], BF16, name="xeb_sb")
    for u in range(2):
        for quarter in range(4):
            nc.sync.dma_start(
                xeb_sb[:, u, ds(1024 * quarter, 1024)],
                xeb_d[:, u, ds(1024 * quarter, 1024)],
            )
    xqf_sb = singles.tile([128, 2, NH], F32, name="xqf_sb")
    nc.sync.dma_start(xqf_sb, xqf_d)

    ones_sb = singles.tile([128, 128], BF16, name="ones_sb")
    nc.vector.memset(ones_sb, 1.0)

    # warm the Exp ACT table during the production phase (table load ~2.7us)
    act_warm = singles.tile([1, 1], F32, name="act_warm")
    nc.scalar.activation(out=act_warm, in_=bq_sb[0:1, :], func=AF.Exp)

    # qT4: qT replicated at 4 partition strips (strip i holds qT[o, :] at
    # partitions 32i+o) for the row-packed / replicated energyT matmuls.
    qT4_sb = singles.tile([128, NH], BF16, name="qT4_sb")
    # k4: k' distributed over strips: strip i = m in [1024i, 1024(i+1))
    assert energy_mode == "pack4", "col-tiled k production supports pack4 only"
    k4_sb = singles.tile([128, 8 * 128], BF16, name="k4_sb")
    vT_sb = singles.tile([128, 32 * C], BF16, name="vT_sb")

    # ---- productions ----
    with tc.tile_pool(name="prodpsum", bufs=2, space="PSUM") as prodpsum:
        # qT4[32i+o, n] = sum_c Wq[o, c] x[c, n] + bq[o]   (wq host-tiled x4)
        for g4 in range(NG):
            psq = prodpsum.tile([128, 512], F32, name="psq")
            for u in range(2):
                nc.tensor.matmul(
                    psq,
                    lhsT=wq_sb[:, u, :],
                    rhs=xqb_sb[:, u, ds(512 * g4, 512)],
                    start=(u == 0),
                    stop=(u == 1),
                )
            nc.vector.tensor_scalar_add(qT4_sb[:, ds(512 * g4, 512)], psq, bq_sb)
        # k'[o, m] = sum_c Wk[o, c] xe[c, m] + kbias[o, m], produced
        # col-tiled (4 concurrent partition strips) directly in strip layout
        psk4 = prodpsum.tile([128, 8 * 128], F32, name="psk4", bufs=1)
        for c8 in range(N // 512):
            i, j = c8 // 2, c8 % 2
            for u in range(2):
                nc.tensor.matmul(
                    psk4[ds(C8 * i, C8), ds(512 * j, 512)],
                    lhsT=wk_sb[:, ds(C8 * u, C8)],
                    rhs=xeb_sb[:, u, ds(512 * c8, 512)],
                    start=(u == 0),
                    stop=(u == 1),
                    tile_position=(0, C8 * i),
                )
        nc.vector.tensor_add(k4_sb, psk4, kbias_sb)
        # vT[m-chunk mc][mm, co] = sum_c xe[c, 128*mc+mm] Wv[co, c]
        def emit_vt_chunk(pool, mc):
            psv = pool.tile([128, C], F32, name="psv")
            for u in range(2):
                nc.tensor.matmul(
                    psv,
                    lhsT=xeb_sb[:, u, ds(128 * mc, 128)],
                    rhs=wv_sb[:, ds(C * u, C)],
                    start=(u == 0),
                    stop=(u == 1),
                )
            nc.vector.tensor_copy(vT_sb[:, ds(C * mc, C)], psv)

        for mc in range(32):
            emit_vt_chunk(prodpsum, mc)

    # ---- main loop: intra-group pipeline with 2-J-step lag.
    # energyT round J: 4 row-packed K=32 matmuls (strips i=0..3) -> 4 psum
    # banks; exp writes pT slots 4J..4J+3. PV of slots 4(J-2).. runs two
    # J-steps behind so PE has work while ACT drains exp. The s-reduction
    # (DVE tree 32->8 chunks + 8 partition-sum matmuls) and the epilogue of
    # group g overlap group g+1's pipeline fill.
    ppool = ctx.enter_context(tc.tile_pool(name="ppool", bufs=2))
    epool = ctx.enter_context(tc.tile_pool(name="epool", bufs=1, space="PSUM"))
    work = ctx.enter_context(tc.tile_pool(name="work", bufs=wbufs))
    mpsum = ctx.enter_context(tc.tile_pool(name="mpsum", bufs=1, space="PSUM"))
    # PSUM budget: epool 4 + pv0/pv1/s_ps 3 = 7 of 8 banks

    def slot_to_chunk(s):
        return 8 * (s % 4) + s // 4 if energy_mode == "pack4" else s

    LAG = lag

    def finish(p):
        """Tree-tail + s-matmuls + normalize + residual + store for group p."""
        gp = p["g"]
        s_ps = mpsum.tile([128, 512], F32, name="s_ps")
        if skip_s:
            nc.vector.memset(s_ps, 1.0)
        else:
            st_b = work.tile([128, 8, 512], BF16, name="st_b", bufs=1)
            nc.vector.tensor_add(
                st_b, p["pT"][:, 16:24, :], p["pT"][:, 24:32, :]
            )
            st_c = work.tile([128, 8, 512], BF16, name="st_c", bufs=1)
            nc.vector.tensor_add(st_c, p["st_a"], st_b)
            st3 = work.tile([128, 4, 512], BF16, name="st3", bufs=1)
            nc.vector.tensor_add(st3, st_c[:, 0:4, :], st_c[:, 4:8, :])
            for s8 in range(4):
                nc.tensor.matmul(
                    s_ps,
                    lhsT=ones_sb,
                    rhs=st3[:, s8, :],
                    start=(s8 == 0),
                    stop=(s8 == 3),
                )
        r_rep = work.tile([128, 512], F32, name="r_rep")
        if skip_recip:
            nc.vector.memset(r_rep, 1.0)
        else:
            nc.vector.reciprocal(r_rep, s_ps)
        for u, ou in enumerate([p["o0"], p["o1"]]):
            t = work.tile([128, 512], F32, name="t")
            nc.vector.scalar_tensor_tensor(
                out=t,
                in0=ou,
                scalar=consts_sb[:, 0:1],
                in1=r_rep,
                op0=OP.mult,
                op1=OP.mult,
            )
            yt = work.tile([128, 512], F32, name="yt")
            nc.vector.scalar_tensor_tensor(
                out=yt,
                in0=t,
                scalar=consts_sb[:, u + 1 : u + 2],
                in1=xqf_sb[:, u, ds(512 * gp, 512)],
                op0=OP.add,
                op1=OP.add,
            )
            nc.sync.dma_start(y_v[:, u, ds(512 * gp, 512)], yt)

    pending = None
    for g in range(NG):
        pT = ppool.tile([128, 32, 512], BF16, name="pT")
        pv0 = mpsum.tile([128, 512], F32, name="pv0")
        pv1 = mpsum.tile([128, 512], F32, name="pv1")
        st_a = None
        def emit_e(J):
                if eps_split:
                    e_lo = epool.tile([128, 2, 512], F32, name="e_lo")
                    e_hi = epool.tile([128, 2, 512], F32, name="e_hi")
                    halves = [e_lo, e_hi]
                    e_ps = None
                else:
                    e_ps = epool.tile([128, 4, 512], F32, name="e_ps")
                for i in range(4):
                    e_out = (
                        halves[i // 2][:, i % 2, :] if eps_split else e_ps[:, i, :]
                    )
                    nc.tensor.matmul(
                        e_out,
                        lhsT=k4_sb[ds(C8 * i, C8), ds(128 * J, 128)],
                        rhs=qT4_sb[ds(C8 * i, C8), ds(512 * g, 512)],
                        start=True,
                        stop=True,
                        tile_position=(C8 * i, 0),
                    )
                escale = 0.25 if energy_mode == "rep128" else 1.0
                if eps_split:
                    for h in range(2):
                        nc.scalar.activation(
                            out=pT[:, ds(4 * J + 2 * h, 2), :],
                            in_=halves[h],
                            func=AF.Exp,
                            scale=escale,
                        )
                else:
                    nc.scalar.activation(
                        out=pT[:, ds(4 * J, 4), :],
                        in_=e_ps,
                        func=AF.Exp,
                        scale=escale,
                    )

        def emit_pv(J):
                for j in range(4):
                    s = 4 * (J - LAG) + j
                    mc = slot_to_chunk(s)
                    st = s == 0
                    sp = s == 31
                    nc.tensor.matmul(
                        pv0,
                        lhsT=vT_sb[:, ds(C * mc, 128)],
                        rhs=pT[:, s, :],
                        start=st,
                        stop=sp,
                    )
                    nc.tensor.matmul(
                        pv1,
                        lhsT=vT_sb[:, ds(C * mc + 128, 128)],
                        rhs=pT[:, s, :],
                        start=st,
                        stop=sp,
                    )

        for J in range(8 + LAG):
            if pv_first:
                if J >= LAG:
                    emit_pv(J)
                if J < 8:
                    emit_e(J)
            else:
                if J < 8:
                    emit_e(J)
                if J >= LAG:
                    emit_pv(J)
            if J == 4 and not skip_s:
                # first half of the s slot-tree: slots 0..15 are ready
                st_a = work.tile([128, 8, 512], BF16, name="st_a")
                nc.vector.tensor_add(st_a, pT[:, 0:8, :], pT[:, 8:16, :])
            if J == finish_at and pending is not None:
                finish(pending)
                pending = None
        # evacuate PV psum to SBUF right away so the psum banks free for the
        # next group's PV; the tree-tail + s-matmuls + normalize/epilogue are
        # deferred into the next group's J-loop (see finish())
        o0 = work.tile([128, 512], F32, name="o0")
        nc.vector.tensor_copy(o0, pv0)
        o1 = work.tile([128, 512], F32, name="o1")
        nc.vector.tensor_copy(o1, pv1)
        pending = dict(g=g, pT=pT, o0=o0, o1=o1, st_a=st_a)
    finish(pending)


INPUT_SPECS = [
    ("xqf", [128, 2 * NH], F32),
    ("xqb", [128, 2 * NH], BF16),
    ("xeb", [128, 2 * N], BF16),
    ("wq", [128, 2 * 128], BF16),
    ("wk", [128, 2 * C8], BF16),
    ("wv", [128, 2 * C], BF16),
    ("kbias", [128, 8 * 128], BF16),
    ("bq", [128, 1], F32),
    ("consts", [128, 3], F32),
]


def _get_program(loop_iters=None, version=2, **opts):
    """loop_iters=None: plain program. loop_iters=k: whole kernel wrapped in a
    device-side For_i loop (for HW timing: slope between two loop counts)."""
    key = ("nc", loop_iters, version, tuple(sorted(opts.items())))
    if key not in _CACHE:
        build = build_attention_v2 if version == 2 else build_attention
        nc = bacc.Bacc("TRN2", debug=False, num_devices=NCORES)
        with tile.TileContext(nc) as tc:
            with ExitStack() as ctx:
                ins = {
                    name: nc.dram_tensor(name, shape, dt, kind="ExternalInput").ap()
                    for name, shape, dt in INPUT_SPECS
                }
                y = nc.dram_tensor("y", [128, 2 * NH], F32, kind="ExternalOutput").ap()
                if loop_iters is None:
                    build(ctx, tc, y, ins, **opts)
                else:
                    # hint_engines: body >256 insts/engine, so the back-edge
                    # would otherwise pay an IRAM refetch (~3-4us) per iter
                    with tc.For_i(
                        0,
                        loop_iters,
                        1,
                        hint_engines=(
                            mybir.EngineType.PE,
                            mybir.EngineType.Activation,
                            mybir.EngineType.DVE,
                            mybir.EngineType.Pool,
                        ),
                    ):
                        with ExitStack() as inner:
                            build(inner, tc, y, ins, **opts)
        nc.compile()
        _CACHE[key] = nc
    return _CACHE[key]


class _Runner:
    """Executes the compiled Bass program on 8 cores via PJRT/axon.

    Mirrors bass2jax.run_bass_via_pjrt's multi-core path, but keeps the
    jitted callable so repeated executions don't re-lower, and supports
    chaining `iters` NEFF executions inside one program (each iteration's
    outputs feed the next iteration's output buffers, creating a data
    dependency) so per-execution device time can be measured without
    host dispatch overhead.
    """

    def __init__(self, nc):
        import jax
        from jax.experimental.shard_map import shard_map
        from jax.sharding import Mesh, PartitionSpec
        from concourse import bass2jax

        bass2jax.install_neuronx_cc_hook()
        self.nc = nc
        self.jax = jax
        in_names, out_names, out_avals, zero_outs = [], [], [], []
        partition_name = (
            nc.partition_id_tensor.name if nc.partition_id_tensor else None
        )
        for alloc in nc.m.functions[0].allocations:
            if not isinstance(alloc, mybir.MemoryLocationSet):
                continue
            name = alloc.memorylocations[0].name
            if alloc.kind == "ExternalInput":
                if name != partition_name:
                    in_names.append(name)
            elif alloc.kind == "ExternalOutput":
                out_names.append(name)
                shape = tuple(alloc.tensor_shape)
                dtype = mybir.dt.np(alloc.dtype)
                out_avals.append(jax.core.ShapedArray(shape, dtype))
                zero_outs.append(np.zeros(shape, dtype))
        self.n_params = len(in_names)
        self.n_outs = len(out_avals)
        self.out_names = out_names
        self.out_avals = out_avals
        self.zero_outs = zero_outs
        all_in_names = list(in_names) + list(out_names)
        if partition_name is not None:
            all_in_names.append(partition_name)
        self.in_names = in_names
        self.partition_name = partition_name

        devices = jax.devices()[:NCORES]
        assert len(devices) == NCORES
        mesh = Mesh(np.asarray(devices), ("core",))
        donate = tuple(range(self.n_params, self.n_params + self.n_outs))
        out_avals_t = tuple(out_avals)
        all_in_names_t = tuple(all_in_names)
        out_names_t = tuple(out_names)

        self.mesh = mesh
        self.pspec = PartitionSpec("core")

        def make(donated):
            def _body(*args):
                operands = list(args)
                if partition_name is not None:
                    operands.append(bass2jax.partition_id_tensor())
                outs = bass2jax._bass_exec_p.bind(
                    *operands,
                    out_avals=out_avals_t,
                    in_names=all_in_names_t,
                    out_names=out_names_t,
                    lowering_input_output_aliases=(),
                    sim_require_finite=True,
                    sim_require_nnan=True,
                    nc=nc,
                )
                return tuple(outs)

            in_specs = (PartitionSpec("core"),) * (self.n_params + self.n_outs)
            out_specs = (PartitionSpec("core"),) * self.n_outs
            return jax.jit(
                shard_map(
                    _body,
                    mesh=mesh,
                    in_specs=in_specs,
                    out_specs=out_specs,
                    check_rep=False,
                ),
                donate_argnums=donate if donated else (),
                keep_unused=True,
            )

        self._make = make
        self._fns = {}

    def _fn(self, donated):
        if donated not in self._fns:
            self._fns[donated] = self._make(donated)
        return self._fns[donated]

    def _concat_args(self, in_maps):
        concat_in = [
            np.concatenate([np.asarray(m[name]) for m in in_maps], axis=0)
            for name in self.in_names
        ]
        concat_zeros = [
            np.zeros((NCORES * z.shape[0], *z.shape[1:]), z.dtype)
            for z in self.zero_outs
        ]
        return concat_in + concat_zeros

    def device_args(self, in_maps):
        """Pre-place sharded args on the 8 devices (for re-execution timing)."""
        jax = self.jax
        from jax.sharding import NamedSharding

        sharding = NamedSharding(self.mesh, self.pspec)
        return [jax.device_put(a, sharding) for a in self._concat_args(in_maps)]

    def execute(self, dev_args):
        """Run on pre-placed device args without donation; returns jax arrays."""
        return self._fn(False)(*dev_args)

    def run(self, in_maps):
        out_arrs = self._fn(True)(*self._concat_args(in_maps))
        out_arrs = [np.asarray(a) for a in out_arrs]
        return [
            {
                name: out_arrs[i].reshape(NCORES, *self.out_avals[i].shape)[c]
                for i, name in enumerate(self.out_names)
            }
            for c in range(NCORES)
        ]


def get_runner():
    if "runner" not in _CACHE:
        _CACHE["runner"] = _Runner(_get_program())
    return _CACHE["runner"]


def get_loop_runner(loop_iters, **opts):
    key = ("runner", loop_iters, tuple(sorted(opts.items())))
    if key not in _CACHE:
        _CACHE[key] = _Runner(_get_program(loop_iters, **opts))
    return _CACHE[key]


def measure_hw_ns(in_maps, k_lo=1, k_hi=129, reps=6, **opts):
    """Per-iteration device time via two For_i loop-count variants."""
    import time as _time
    import jax as _jax

    def bench(runner):
        dev = runner.device_args(in_maps)
        for _ in range(2):
            _jax.block_until_ready(runner.execute(dev))
        best = float("inf")
        for _ in range(reps):
            t0 = _time.perf_counter()
            _jax.block_until_ready(runner.execute(dev))
            best = min(best, _time.perf_counter() - t0)
        return best

    t_lo = bench(get_loop_runner(k_lo, **opts))
    t_hi = bench(get_loop_runner(k_hi, **opts))
    return (t_hi - t_lo) / (k_hi - k_lo) * 1e9, t_lo, t_hi


def get_trivial_runner():
    """Near-empty NEFF (one tiny DMA in->out) to measure dispatch overhead."""
    if "trivial" not in _CACHE:
        nc = bacc.Bacc("TRN2", debug=False, num_devices=NCORES)
        with tile.TileContext(nc) as tc:
            with ExitStack() as ctx:
                tin = nc.dram_tensor("tin", [128, 8], F32, kind="ExternalInput").ap()
                tout = nc.dram_tensor(
                    "tout", [128, 8], F32, kind="ExternalOutput"
                ).ap()
                pool = ctx.enter_context(tc.tile_pool(name="tpool", bufs=1))
                tt = pool.tile([128, 8], F32, name="tt")
                nc.sync.dma_start(tt, tin)
                nc.sync.dma_start(tout, tt)
        nc.compile()
        _CACHE["trivial"] = _Runner(nc)
    return _CACHE["trivial"]


def _to2(a):
    """[256, X] -> [128, 2X] with out[p, u*X + j] = a[128u + p, j]."""
    x = np.asarray(a)
    return np.ascontiguousarray(
        x.reshape(2, 128, x.shape[1]).transpose(1, 0, 2).reshape(128, -1)
    )


def _bf(a):
    return np.ascontiguousarray(np.asarray(a, dtype=ml_dtypes.bfloat16))


def _f32(a):
    return np.ascontiguousarray(np.asarray(a, dtype=np.float32))


def kernel(x, x_encoder, Wq, bq, Wk, bk, Wv, bv, h_pos, w_pos, gamma):
    global LAST_EXEC_TIME_NS
    in_maps = make_in_maps(
        x, x_encoder, Wq, bq, Wk, bk, Wv, bv, h_pos, w_pos, gamma
    )
    runner = get_runner()
    results = runner.run(in_maps)

    out = np.empty((B, C, N), np.float32)
    for core in range(NCORES):
        b, half = divmod(core, 2)
        yc = results[core]["y"]  # [128, 2*NH]
        out[b][:, half * NH : (half + 1) * NH] = (
            yc.reshape(128, 2, NH).transpose(1, 0, 2).reshape(C, NH)
        )
    return out.reshape(B, C, H, W)


def make_in_maps(x, x_encoder, Wq, bq, Wk, bk, Wv, bv, h_pos, w_pos, gamma):
    """Host-side input prep shared by kernel() and timing harnesses."""
    x = _f32(x)
    x_encoder = _f32(x_encoder)
    Wq, bq, Wk, bk, Wv, bv = map(_f32, (Wq, bq, Wk, bk, Wv, bv))
    h_pos, w_pos, gamma = map(_f32, (h_pos, w_pos, gamma))
    xf = x.reshape(B, C, N)
    xe = x_encoder.reshape(B, C, N)
    pos = (h_pos + w_pos).reshape(C8, N)
    kb = bk[:, None] + pos  # [32, 4096]
    # strip layout: kbias4[32i+o, j] = kb[o, 1024i + j]
    kbias = _bf(kb.reshape(C8, 4, 8 * 128).transpose(1, 0, 2).reshape(128, 8 * 128))
    wqT = Wq.T  # [256, 32]
    wq_h = _bf(
        np.concatenate(
            [np.tile(wqT[128 * u : 128 * (u + 1)], (1, 4)) for u in range(2)],
            axis=1,
        )
    )  # [128, 256]: wq_h[p, 128u + 32i + o] = Wq[o, 128u + p]
    wk_h = _bf(_to2(Wk.T))
    wv_h = _bf(_to2(Wv.T))
    bq_h = _f32(np.tile(bq, 4)[:, None])  # [128, 1]
    g = float(gamma.reshape(-1)[0])
    consts = np.empty((128, 3), np.float32)
    consts[:, 0] = g
    consts[:, 1] = g * bv[0:128]
    consts[:, 2] = g * bv[128:256]
    in_maps = []
    for core in range(NCORES):
        b, half = divmod(core, 2)
        xq = _to2(xf[b][:, half * NH : (half + 1) * NH])
        in_maps.append(
            {
                "xqf": _f32(xq),
                "xqb": _bf(xq),
                "xeb": _bf(_to2(xe[b])),
                "wq": wq_h,
                "wk": wk_h,
                "wv": wv_h,
                "kbias": kbias,
                "bq": bq_h,
                "consts": consts,
            }
        )
    return in_maps


if __name__ == "__main__":
    import reference

    inputs = {k: np.asarray(v) for k, v in reference.setup_inputs().items()}
    got = kernel(**inputs)
    print("kernel ran; output shape", got.shape, "exec_ns", LAST_EXEC_TIME_NS)



# revision 9
# speedup vs baseline: 1.1368x; 1.1322x over previous
"""Trainium2 Bass kernel for nn_AttentionDecoder (B=4, C=256, H=W=64).

Math (per batch b):
    q  = Wq @ x[b]  + bq          [32, N]   (as qT on device: [32, N] with o on partitions)
    k' = Wk @ xe[b] + bk + pos    [32, N]
    v  = Wv @ xe[b]               [256, N]  (bv folded into epilogue: sum(attn)=1)
    eT = k'^T-chunks: energy^T[m, n] = sum_o k'[o,m] q[o,n]
    pT = exp(eT)                  (no max-subtraction: |energy| < ~30, fp32-exp safe)
    out[c, n] = sum_m v[c, m] pT[m, n]      (PE: lhsT=vT chunk, rhs=pT chunk)
    s[n]      = sum_m pT[m, n]              (PE: lhsT=ones -> replicated rows)
    y = gamma * (out / s + bv) + x

Sharding: 8 cores = (batch, query-half). Each core: 2048 query rows, full m=4096.
"""

import numpy as np
import ml_dtypes
from contextlib import ExitStack

import concourse.bass as bass
import concourse.bass_isa as bass_isa
import concourse.bacc as bacc
import concourse.tile as tile
import concourse.mybir as mybir
from concourse.bass import ds, ts

B, C, H, W = 4, 256, 64, 64
N = H * W          # 4096
C8 = 32
NH = N // 2        # 2048 query rows per core
NCORES = 8
NG = NH // 512     # 4 n-groups of 512 per core
F32 = mybir.dt.float32
BF16 = mybir.dt.bfloat16
AF = mybir.ActivationFunctionType
OP = mybir.AluOpType

LAST_EXEC_TIME_NS = None
_CACHE = {}


def build_attention_v2(
    ctx,
    tc,
    y,
    ins,
    lag=2,
    vt_act_share=2,
    finish_at=0,
    st_a_pool=False,
    s_allreduce="pool",
    pool_dma=False,
):
    """v2: stall-free PE schedule.

    - energy psum double-buffered as [128,2,512] pairs (4 banks), exp in
      2-slot instructions so round J+1's energy never waits a 4-slot drain
    - pv psum [128,2,512] double-buffered across groups (4 banks); the
      epilogue STTs read pv psum directly (no o0/o1 evacuation)
    - s-reduction: slot-tree split across Pool (st_a) and DVE, reduced to
      [128,512] with only the last-8-slot fold on the tail critical path;
      final cross-partition sum via gpsimd partition_all_reduce (no PE
      s-matmuls, no s psum bank)
    - input DMA split across SP and Pool queues in consumption order;
      vT psum evacuation split DVE/ACT; q bias-add on ACT (Copy+bias)
    """
    nc = tc.nc
    y_v = y.rearrange("p (u n) -> p u n", u=2)
    xqf_d = ins["xqf"].rearrange("p (u n) -> p u n", u=2)
    xqb_d = ins["xqb"].rearrange("p (u n) -> p u n", u=2)
    xeb_d = ins["xeb"].rearrange("p (u m) -> p u m", u=2)

    singles = ctx.enter_context(tc.tile_pool(name="singles", bufs=1))

    # ---- resident SBUF tensors ----
    wq_sb = singles.tile([128, 2, 128], BF16, name="wq_sb")
    bq_sb = singles.tile([128, 1], F32, name="bq_sb")
    wk_sb = singles.tile([128, 2 * C8], BF16, name="wk_sb")
    kbias_sb = singles.tile([128, 8 * 128], BF16, name="kbias_sb")
    wv_sb = singles.tile([128, 2 * C], BF16, name="wv_sb")
    consts_sb = singles.tile([128, 3], F32, name="consts_sb")
    xqb_sb = singles.tile([128, 2, NH], BF16, name="xqb_sb")
    xeb_sb = singles.tile([128, 2, N], BF16, name="xeb_sb")
    xqf_sb = singles.tile([128, 2, NH], F32, name="xqf_sb")
    qT4_sb = singles.tile([128, NH], BF16, name="qT4_sb")
    k4_sb = singles.tile([128, 8 * 128], BF16, name="k4_sb")
    vT_sb = singles.tile([128, 32 * C], BF16, name="vT_sb")

    # SP DMA queue, ordered to match PE consumption: q prod, k prod, vT prod
    nc.sync.dma_start(wq_sb, ins["wq"].rearrange("p (u j) -> p u j", u=2))
    nc.sync.dma_start(bq_sb, ins["bq"])
    for u in range(2):
        nc.sync.dma_start(xqb_sb[:, u, ds(0, 512)], xqb_d[:, u, ds(0, 512)])
    nc.sync.dma_start(wk_sb, ins["wk"])
    for quarter in range(4):
        for u in range(2):
            nc.sync.dma_start(
                xeb_sb[:, u, ds(1024 * quarter, 1024)],
                xeb_d[:, u, ds(1024 * quarter, 1024)],
            )
        if quarter == 0:
            nc.sync.dma_start(kbias_sb, ins["kbias"])
    # Pool DMA queue: weights/residual not needed until later; Pool engine
    # is idle until the first st_a (~20us in)
    dma2 = nc.gpsimd.dma_start if pool_dma else nc.sync.dma_start
    dma2(wv_sb, ins["wv"])
    dma2(consts_sb, ins["consts"])
    for g4 in range(1, NG):
        for u in range(2):
            dma2(xqb_sb[:, u, ds(512 * g4, 512)], xqb_d[:, u, ds(512 * g4, 512)])
    dma2(xqf_sb, xqf_d)

    # warm the Exp ACT table during the production phase (table load ~2.7us)
    act_warm = singles.tile([1, 1], F32, name="act_warm")
    nc.scalar.activation(out=act_warm, in_=bq_sb[0:1, :], func=AF.Exp)

    # ---- productions ----
    vT32 = vT_sb.rearrange("p (d c) -> p d c", c=C)
    with tc.tile_pool(name="psqp", bufs=2, space="PSUM") as psqp, tc.tile_pool(
        name="pskp", bufs=1, space="PSUM"
    ) as pskp, tc.tile_pool(name="psvp", bufs=3, space="PSUM") as psvp:

        def emit_q(g4):
            psq = psqp.tile([128, 512], F32, name="psq")
            for u in range(2):
                nc.tensor.matmul(
                    psq,
                    lhsT=wq_sb[:, u, :],
                    rhs=xqb_sb[:, u, ds(512 * g4, 512)],
                    start=(u == 0),
                    stop=(u == 1),
                )
            nc.scalar.activation(
                out=qT4_sb[:, ds(512 * g4, 512)], in_=psq, func=AF.Identity, bias=bq_sb
            )

        psk4 = pskp.tile([128, 8 * 128], F32, name="psk4")

        def emit_k(c8):
            i, j = c8 // 2, c8 % 2
            for u in range(2):
                nc.tensor.matmul(
                    psk4[ds(C8 * i, C8), ds(512 * j, 512)],
                    lhsT=wk_sb[:, ds(C8 * u, C8)],
                    rhs=xeb_sb[:, u, ds(512 * c8, 512)],
                    start=(u == 0),
                    stop=(u == 1),
                    tile_position=(0, C8 * i),
                )

        def emit_vt(t):
            # pair of m-positions (2t, 2t+1); evac alternates DVE/ACT
            psv = psvp.tile([128, 2, C], F32, name="psv")
            for w in range(2):
                for u in range(2):
                    nc.tensor.matmul(
                        psv[:, w, :],
                        lhsT=xeb_sb[:, u, ds(128 * (2 * t + w), 128)],
                        rhs=wv_sb[:, ds(C * u, C)],
                        start=(u == 0),
                        stop=(u == 1),
                    )
            dest = vT32[:, ds(2 * t, 2), :]
            if vt_act_share and t % vt_act_share == 0:
                nc.scalar.copy(dest, psv)
            else:
                nc.vector.tensor_copy(dest, psv)

        for g4 in range(NG):
            emit_q(g4)
        emit_k(0)
        emit_k(1)
        emit_vt(0)
        emit_vt(1)
        emit_k(2)
        emit_k(3)
        emit_vt(2)
        emit_vt(3)
        emit_vt(4)
        emit_vt(5)
        emit_k(4)
        emit_k(5)
        emit_vt(6)
        emit_vt(7)
        emit_k(6)
        emit_k(7)
        nc.vector.tensor_add(k4_sb, psk4, kbias_sb)
        for t in range(8, 16):
            emit_vt(t)

    # ---- main loop ----
    ppool = ctx.enter_context(tc.tile_pool(name="ppool", bufs=2))
    epool = ctx.enter_context(tc.tile_pool(name="epool", bufs=2, space="PSUM"))
    mpsum = ctx.enter_context(tc.tile_pool(name="mpsum", bufs=2, space="PSUM"))
    work = ctx.enter_context(tc.tile_pool(name="work", bufs=2))
    # PSUM budget: epool 2 tiles x 2 banks + mpsum 2 tiles x 2 banks = 8

    def slot_to_chunk(s):
        return 8 * (s % 4) + s // 4

    LAG = lag

    def finish(p):
        """Tail-8-slot fold + s + normalize + residual + store for group p
        (runs during the next group's early rounds; reads pv psum)."""
        gp = p["g"]
        b2 = work.tile([128, 4, 512], BF16, name="b2")
        nc.vector.tensor_add(b2, p["pT"][:, 24:28, :], p["pT"][:, 28:32, :])
        b2a = work.tile([128, 2, 512], BF16, name="b2a")
        nc.vector.tensor_add(b2a, b2[:, 0:2, :], b2[:, 2:4, :])
        b2b = work.tile([128, 512], BF16, name="b2b")
        nc.vector.tensor_add(b2b, b2a[:, 0, :], b2a[:, 1, :])
        st = work.tile([128, 512], BF16, name="st")
        nc.vector.tensor_add(st, p["c3"], b2b)
        s_rep = work.tile([128, 512], F32, name="s_rep")
        if s_allreduce == "pool":
            nc.gpsimd.partition_all_reduce(
                s_rep, st, channels=128, reduce_op=bass_isa.ReduceOp.add
            )
        else:  # timing-only stub (wrong s, gamma=0 output unaffected)
            nc.vector.memset(s_rep, 1.0)
        r_rep = work.tile([128, 512], F32, name="r_rep")
        nc.vector.reciprocal(r_rep, s_rep)
        for u in range(2):
            t = work.tile([128, 512], F32, name="t")
            nc.vector.scalar_tensor_tensor(
                out=t,
                in0=p["pv"][:, u, :],
                scalar=consts_sb[:, 0:1],
                in1=r_rep,
                op0=OP.mult,
                op1=OP.mult,
            )
            yt = work.tile([128, 512], F32, name="yt")
            nc.vector.scalar_tensor_tensor(
                out=yt,
                in0=t,
                scalar=consts_sb[:, u + 1 : u + 2],
                in1=xqf_sb[:, u, ds(512 * gp, 512)],
                op0=OP.add,
                op1=OP.add,
            )
            nc.sync.dma_start(y_v[:, u, ds(512 * gp, 512)], yt)

    pending = None
    for g in range(NG):
        pT = ppool.tile([128, 32, 512], BF16, name="pT")
        pv = mpsum.tile([128, 2, 512], F32, name="pv")
        cur = {"g": g, "pT": pT, "pv": pv}
        for J in range(8 + LAG):
            if J < 8:
                for h in range(2):
                    e2 = epool.tile([128, 2, 512], F32, name="e2")
                    for k in range(2):
                        i = 2 * h + k
                        nc.tensor.matmul(
                            e2[:, k, :],
                            lhsT=k4_sb[ds(C8 * i, C8), ds(128 * J, 128)],
                            rhs=qT4_sb[ds(C8 * i, C8), ds(512 * g, 512)],
                            start=True,
                            stop=True,
                            tile_position=(C8 * i, 0),
                        )
                    nc.scalar.activation(
                        out=pT[:, ds(4 * J + 2 * h, 2), :], in_=e2, func=AF.Exp
                    )
            if J >= LAG and J - LAG < 8:
                for j in range(4):
                    s = 4 * (J - LAG) + j
                    mc = slot_to_chunk(s)
                    st = s == 0
                    sp = s == 31
                    for u in range(2):
                        nc.tensor.matmul(
                            pv[:, u, :],
                            lhsT=vT_sb[:, ds(C * mc + 128 * u, 128)],
                            rhs=pT[:, s, :],
                            start=st,
                            stop=sp,
                        )
            if J == 4:
                # Pool is otherwise idle: big first-level fold of slots 0:16
                cur["st_a"] = work.tile([128, 8, 512], BF16, name="st_a")
                (nc.gpsimd if st_a_pool else nc.vector).tensor_add(
                    cur["st_a"], pT[:, 0:8, :], pT[:, 8:16, :]
                )
            if J == 5:
                cur["b1"] = work.tile([128, 4, 512], BF16, name="b1")
                nc.vector.tensor_add(cur["b1"], pT[:, 16:20, :], pT[:, 20:24, :])
            if J == 6:
                cur["a2"] = work.tile([128, 4, 512], BF16, name="a2")
                nc.vector.tensor_add(
                    cur["a2"], cur["st_a"][:, 0:4, :], cur["st_a"][:, 4:8, :]
                )
                cur["cc"] = work.tile([128, 4, 512], BF16, name="cc")
                nc.vector.tensor_add(cur["cc"], cur["a2"], cur["b1"])
            if J == 7:
                cur["c2"] = work.tile([128, 2, 512], BF16, name="c2")
                nc.vector.tensor_add(
                    cur["c2"], cur["cc"][:, 0:2, :], cur["cc"][:, 2:4, :]
                )
                cur["c3"] = work.tile([128, 512], BF16, name="c3")
                nc.vector.tensor_add(cur["c3"], cur["c2"][:, 0, :], cur["c2"][:, 1, :])
            if J == finish_at and pending is not None:
                finish(pending)
                pending = None
        pending = cur
    finish(pending)


def build_attention(
    ctx,
    tc,
    y,
    ins,
    energy_mode="pack4",
    skip_s=False,
    lag=2,
    tree_levels=3,
    eps_split=False,
    wbufs=2,
    skip_recip=False,
    pv_first=False,
    finish_at=2,
):
    """y: [128, 2*NH] f32 dram AP.  ins: dict of dram APs (see kernel()).

    energy_mode:
      'pack4'  - 4x row-tiled K=32 matmuls (tile_position), k' strip-split
      'k32'    - plain K=32 matmuls at partitions 0:32 (no tile_position)
      'rep128' - k' replicated on all 4 strips, full K=128 matmul computes
                 4x energy; the 1/4 is folded into exp's scale (free affine)
    """
    nc = tc.nc
    y_v = y.rearrange("p (u n) -> p u n", u=2)
    xqf_d = ins["xqf"].rearrange("p (u n) -> p u n", u=2)
    xqb_d = ins["xqb"].rearrange("p (u n) -> p u n", u=2)
    xeb_d = ins["xeb"].rearrange("p (u m) -> p u m", u=2)

    singles = ctx.enter_context(tc.tile_pool(name="singles", bufs=1))

    # ---- resident SBUF tensors; DMA order matters: what productions need
    # first goes first, the residual input (only needed ~60us in) goes last.
    wq_sb = singles.tile([128, 2, 128], BF16, name="wq_sb")
    nc.sync.dma_start(wq_sb, ins["wq"].rearrange("p (u j) -> p u j", u=2))
    wk_sb = singles.tile([128, 2 * C8], BF16, name="wk_sb")
    nc.sync.dma_start(wk_sb, ins["wk"])
    wv_sb = singles.tile([128, 2 * C], BF16, name="wv_sb")
    nc.sync.dma_start(wv_sb, ins["wv"])
    bq_sb = singles.tile([128, 1], F32, name="bq_sb")
    nc.sync.dma_start(bq_sb, ins["bq"])
    kbias_sb = singles.tile([128, 8 * 128], BF16, name="kbias_sb")
    nc.sync.dma_start(kbias_sb, ins["kbias"])
    consts_sb = singles.tile([128, 3], F32, name="consts_sb")
    nc.sync.dma_start(consts_sb, ins["consts"])
    # xqb split per (c-chunk, n-group) so qT production starts on first chunk
    xqb_sb = singles.tile([128, 2, NH], BF16, name="xqb_sb")
    for g4 in range(NG):
        for u in range(2):
            nc.sync.dma_start(
                xqb_sb[:, u, ds(512 * g4, 512)], xqb_d[:, u, ds(512 * g4, 512)]
            )
    # xeb split into chunks so k/v production starts before the full 2MB lands
    xeb_sb = singles.tile([128, 2, N], BF16, name="xeb_sb")
    for u in range(2):
        for quarter in range(4):
            nc.sync.dma_start(
                xeb_sb[:, u, ds(1024 * quarter, 1024)],
                xeb_d[:, u, ds(1024 * quarter, 1024)],
            )
    xqf_sb = singles.tile([128, 2, NH], F32, name="xqf_sb")
    nc.sync.dma_start(xqf_sb, xqf_d)

    ones_sb = singles.tile([128, 128], BF16, name="ones_sb")
    nc.vector.memset(ones_sb, 1.0)

    # warm the Exp ACT table during the production phase (table load ~2.7us)
    act_warm = singles.tile([1, 1], F32, name="act_warm")
    nc.scalar.activation(out=act_warm, in_=bq_sb[0:1, :], func=AF.Exp)

    # qT4: qT replicated at 4 partition strips (strip i holds qT[o, :] at
    # partitions 32i+o) for the row-packed / replicated energyT matmuls.
    qT4_sb = singles.tile([128, NH], BF16, name="qT4_sb")
    # k4: k' distributed over strips: strip i = m in [1024i, 1024(i+1))
    assert energy_mode == "pack4", "col-tiled k production supports pack4 only"
    k4_sb = singles.tile([128, 8 * 128], BF16, name="k4_sb")
    vT_sb = singles.tile([128, 32 * C], BF16, name="vT_sb")

    # ---- productions ----
    with tc.tile_pool(name="prodpsum", bufs=2, space="PSUM") as prodpsum:
        # qT4[32i+o, n] = sum_c Wq[o, c] x[c, n] + bq[o]   (wq host-tiled x4)
        for g4 in range(NG):
            psq = prodpsum.tile([128, 512], F32, name="psq")
            for u in range(2):
                nc.tensor.matmul(
                    psq,
                    lhsT=wq_sb[:, u, :],
                    rhs=xqb_sb[:, u, ds(512 * g4, 512)],
                    start=(u == 0),
                    stop=(u == 1),
                )
            nc.vector.tensor_scalar_add(qT4_sb[:, ds(512 * g4, 512)], psq, bq_sb)
        # k'[o, m] = sum_c Wk[o, c] xe[c, m] + kbias[o, m], produced
        # col-tiled (4 concurrent partition strips) directly in strip layout
        psk4 = prodpsum.tile([128, 8 * 128], F32, name="psk4", bufs=1)
        for c8 in range(N // 512):
            i, j = c8 // 2, c8 % 2
            for u in range(2):
                nc.tensor.matmul(
                    psk4[ds(C8 * i, C8), ds(512 * j, 512)],
                    lhsT=wk_sb[:, ds(C8 * u, C8)],
                    rhs=xeb_sb[:, u, ds(512 * c8, 512)],
                    start=(u == 0),
                    stop=(u == 1),
                    tile_position=(0, C8 * i),
                )
        nc.vector.tensor_add(k4_sb, psk4, kbias_sb)
        # vT[m-chunk mc][mm, co] = sum_c xe[c, 128*mc+mm] Wv[co, c]
        def emit_vt_chunk(pool, mc):
            psv = pool.tile([128, C], F32, name="psv")
            for u in range(2):
                nc.tensor.matmul(
                    psv,
                    lhsT=xeb_sb[:, u, ds(128 * mc, 128)],
                    rhs=wv_sb[:, ds(C * u, C)],
                    start=(u == 0),
                    stop=(u == 1),
                )
            nc.vector.tensor_copy(vT_sb[:, ds(C * mc, C)], psv)

        for mc in range(32):
            emit_vt_chunk(prodpsum, mc)

    # ---- main loop: intra-group pipeline with 2-J-step lag.
    # energyT round J: 4 row-packed K=32 matmuls (strips i=0..3) -> 4 psum
    # banks; exp writes pT slots 4J..4J+3. PV of slots 4(J-2).. runs two
    # J-steps behind so PE has work while ACT drains exp. The s-reduction
    # (DVE tree 32->8 chunks + 8 partition-sum matmuls) and the epilogue of
    # group g overlap group g+1's pipeline fill.
    ppool = ctx.enter_context(tc.tile_pool(name="ppool", bufs=2))
    epool = ctx.enter_context(tc.tile_pool(name="epool", bufs=1, space="PSUM"))
    work = ctx.enter_context(tc.tile_pool(name="work", bufs=wbufs))
    mpsum = ctx.enter_context(tc.tile_pool(name="mpsum", bufs=1, space="PSUM"))
    # PSUM budget: epool 4 + pv0/pv1/s_ps 3 = 7 of 8 banks

    def slot_to_chunk(s):
        return 8 * (s % 4) + s // 4 if energy_mode == "pack4" else s

    LAG = lag

    def finish(p):
        """Tree-tail + s-matmuls + normalize + residual + store for group p."""
        gp = p["g"]
        s_ps = mpsum.tile([128, 512], F32, name="s_ps")
        if skip_s:
            nc.vector.memset(s_ps, 1.0)
        else:
            st_b = work.tile([128, 8, 512], BF16, name="st_b", bufs=1)
            nc.vector.tensor_add(
                st_b, p["pT"][:, 16:24, :], p["pT"][:, 24:32, :]
            )
            st_c = work.tile([128, 8, 512], BF16, name="st_c", bufs=1)
            nc.vector.tensor_add(st_c, p["st_a"], st_b)
            st3 = work.tile([128, 4, 512], BF16, name="st3", bufs=1)
            nc.vector.tensor_add(st3, st_c[:, 0:4, :], st_c[:, 4:8, :])
            for s8 in range(4):
                nc.tensor.matmul(
                    s_ps,
                    lhsT=ones_sb,
                    rhs=st3[:, s8, :],
                    start=(s8 == 0),
                    stop=(s8 == 3),
                )
        r_rep = work.tile([128, 512], F32, name="r_rep")
        if skip_recip:
            nc.vector.memset(r_rep, 1.0)
        else:
            nc.vector.reciprocal(r_rep, s_ps)
        for u, ou in enumerate([p["o0"], p["o1"]]):
            t = work.tile([128, 512], F32, name="t")
            nc.vector.scalar_tensor_tensor(
                out=t,
                in0=ou,
                scalar=consts_sb[:, 0:1],
                in1=r_rep,
                op0=OP.mult,
                op1=OP.mult,
            )
            yt = work.tile([128, 512], F32, name="yt")
            nc.vector.scalar_tensor_tensor(
                out=yt,
                in0=t,
                scalar=consts_sb[:, u + 1 : u + 2],
                in1=xqf_sb[:, u, ds(512 * gp, 512)],
                op0=OP.add,
                op1=OP.add,
            )
            nc.sync.dma_start(y_v[:, u, ds(512 * gp, 512)], yt)

    pending = None
    for g in range(NG):
        pT = ppool.tile([128, 32, 512], BF16, name="pT")
        pv0 = mpsum.tile([128, 512], F32, name="pv0")
        pv1 = mpsum.tile([128, 512], F32, name="pv1")
        st_a = None
        def emit_e(J):
                if eps_split:
                    e_lo = epool.tile([128, 2, 512], F32, name="e_lo")
                    e_hi = epool.tile([128, 2, 512], F32, name="e_hi")
                    halves = [e_lo, e_hi]
                    e_ps = None
                else:
                    e_ps = epool.tile([128, 4, 512], F32, name="e_ps")
                for i in range(4):
                    e_out = (
                        halves[i // 2][:, i % 2, :] if eps_split else e_ps[:, i, :]
                    )
                    nc.tensor.matmul(
                        e_out,
                        lhsT=k4_sb[ds(C8 * i, C8), ds(128 * J, 128)],
                        rhs=qT4_sb[ds(C8 * i, C8), ds(512 * g, 512)],
                        start=True,
                        stop=True,
                        tile_position=(C8 * i, 0),
                    )
                escale = 0.25 if energy_mode == "rep128" else 1.0
                if eps_split:
                    for h in range(2):
                        nc.scalar.activation(
                            out=pT[:, ds(4 * J + 2 * h, 2), :],
                            in_=halves[h],
                            func=AF.Exp,
                            scale=escale,
                        )
                else:
                    nc.scalar.activation(
                        out=pT[:, ds(4 * J, 4), :],
                        in_=e_ps,
                        func=AF.Exp,
                        scale=escale,
                    )

        def emit_pv(J):
                for j in range(4):
                    s = 4 * (J - LAG) + j
                    mc = slot_to_chunk(s)
                    st = s == 0
                    sp = s == 31
                    nc.tensor.matmul(
                        pv0,
                        lhsT=vT_sb[:, ds(C * mc, 128)],
                        rhs=pT[:, s, :],
                        start=st,
                        stop=sp,
                    )
                    nc.tensor.matmul(
                        pv1,
                        lhsT=vT_sb[:, ds(C * mc + 128, 128)],
                        rhs=pT[:, s, :],
                        start=st,
                        stop=sp,
                    )

        for J in range(8 + LAG):
            if pv_first:
                if J >= LAG:
                    emit_pv(J)
                if J < 8:
                    emit_e(J)
            else:
                if J < 8:
                    emit_e(J)
                if J >= LAG:
                    emit_pv(J)
            if J == 4 and not skip_s:
                # first half of the s slot-tree: slots 0..15 are ready
                st_a = work.tile([128, 8, 512], BF16, name="st_a")
                nc.vector.tensor_add(st_a, pT[:, 0:8, :], pT[:, 8:16, :])
            if J == finish_at and pending is not None:
                finish(pending)
                pending = None
        # evacuate PV psum to SBUF right away so the psum banks free for the
        # next group's PV; the tree-tail + s-matmuls + normalize/epilogue are
        # deferred into the next group's J-loop (see finish())
        o0 = work.tile([128, 512], F32, name="o0")
        nc.vector.tensor_copy(o0, pv0)
        o1 = work.tile([128, 512], F32, name="o1")
        nc.vector.tensor_copy(o1, pv1)
        pending = dict(g=g, pT=pT, o0=o0, o1=o1, st_a=st_a)
    finish(pending)


INPUT_SPECS = [
    ("xqf", [128, 2 * NH], F32),
    ("xqb", [128, 2 * NH], BF16),
    ("xeb", [128, 2 * N], BF16),
    ("wq", [128, 2 * 128], BF16),
    ("wk", [128, 2 * C8], BF16),
    ("wv", [128, 2 * C], BF16),
    ("kbias", [128, 8 * 128], BF16),
    ("bq", [128, 1], F32),
    ("consts", [128, 3], F32),
]


def _get_program(loop_iters=None, version=2, **opts):
    """loop_iters=None: plain program. loop_iters=k: whole kernel wrapped in a
    device-side For_i loop (for HW timing: slope between two loop counts)."""
    key = ("nc", loop_iters, version, tuple(sorted(opts.items())))
    if key not in _CACHE:
        build = build_attention_v2 if version == 2 else build_attention
        nc = bacc.Bacc("TRN2", debug=False, num_devices=NCORES)
        with tile.TileContext(nc) as tc:
            with ExitStack() as ctx:
                ins = {
                    name: nc.dram_tensor(name, shape, dt, kind="ExternalInput").ap()
                    for name, shape, dt in INPUT_SPECS
                }
                y = nc.dram_tensor("y", [128, 2 * NH], F32, kind="ExternalOutput").ap()
                if loop_iters is None:
                    build(ctx, tc, y, ins, **opts)
                else:
                    # hint_engines: body >256 insts/engine, so the back-edge
                    # would otherwise pay an IRAM refetch (~3-4us) per iter
                    with tc.For_i(
                        0,
                        loop_iters,
                        1,
                        hint_engines=(
                            mybir.EngineType.PE,
                            mybir.EngineType.Activation,
                            mybir.EngineType.DVE,
                            mybir.EngineType.Pool,
                        ),
                    ):
                        with ExitStack() as inner:
                            build(inner, tc, y, ins, **opts)
        nc.compile()
        _CACHE[key] = nc
    return _CACHE[key]


class _Runner:
    """Executes the compiled Bass program on 8 cores via PJRT/axon.

    Mirrors bass2jax.run_bass_via_pjrt's multi-core path, but keeps the
    jitted callable so repeated executions don't re-lower, and supports
    chaining `iters` NEFF executions inside one program (each iteration's
    outputs feed the next iteration's output buffers, creating a data
    dependency) so per-execution device time can be measured without
    host dispatch overhead.
    """

    def __init__(self, nc):
        import jax
        from jax.experimental.shard_map import shard_map
        from jax.sharding import Mesh, PartitionSpec
        from concourse import bass2jax

        bass2jax.install_neuronx_cc_hook()
        self.nc = nc
        self.jax = jax
        in_names, out_names, out_avals, zero_outs = [], [], [], []
        partition_name = (
            nc.partition_id_tensor.name if nc.partition_id_tensor else None
        )
        for alloc in nc.m.functions[0].allocations:
            if not isinstance(alloc, mybir.MemoryLocationSet):
                continue
            name = alloc.memorylocations[0].name
            if alloc.kind == "ExternalInput":
                if name != partition_name:
                    in_names.append(name)
            elif alloc.kind == "ExternalOutput":
                out_names.append(name)
                shape = tuple(alloc.tensor_shape)
                dtype = mybir.dt.np(alloc.dtype)
                out_avals.append(jax.core.ShapedArray(shape, dtype))
                zero_outs.append(np.zeros(shape, dtype))
        self.n_params = len(in_names)
        self.n_outs = len(out_avals)
        self.out_names = out_names
        self.out_avals = out_avals
        self.zero_outs = zero_outs
        all_in_names = list(in_names) + list(out_names)
        if partition_name is not None:
            all_in_names.append(partition_name)
        self.in_names = in_names
        self.partition_name = partition_name

        devices = jax.devices()[:NCORES]
        assert len(devices) == NCORES
        mesh = Mesh(np.asarray(devices), ("core",))
        donate = tuple(range(self.n_params, self.n_params + self.n_outs))
        out_avals_t = tuple(out_avals)
        all_in_names_t = tuple(all_in_names)
        out_names_t = tuple(out_names)

        self.mesh = mesh
        self.pspec = PartitionSpec("core")

        def make(donated):
            def _body(*args):
                operands = list(args)
                if partition_name is not None:
                    operands.append(bass2jax.partition_id_tensor())
                outs = bass2jax._bass_exec_p.bind(
                    *operands,
                    out_avals=out_avals_t,
                    in_names=all_in_names_t,
                    out_names=out_names_t,
                    lowering_input_output_aliases=(),
                    sim_require_finite=True,
                    sim_require_nnan=True,
                    nc=nc,
                )
                return tuple(outs)

            in_specs = (PartitionSpec("core"),) * (self.n_params + self.n_outs)
            out_specs = (PartitionSpec("core"),) * self.n_outs
            return jax.jit(
                shard_map(
                    _body,
                    mesh=mesh,
                    in_specs=in_specs,
                    out_specs=out_specs,
                    check_rep=False,
                ),
                donate_argnums=donate if donated else (),
                keep_unused=True,
            )

        self._make = make
        self._fns = {}

    def _fn(self, donated):
        if donated not in self._fns:
            self._fns[donated] = self._make(donated)
        return self._fns[donated]

    def _concat_args(self, in_maps):
        concat_in = [
            np.concatenate([np.asarray(m[name]) for m in in_maps], axis=0)
            for name in self.in_names
        ]
        concat_zeros = [
            np.zeros((NCORES * z.shape[0], *z.shape[1:]), z.dtype)
            for z in self.zero_outs
        ]
        return concat_in + concat_zeros

    def device_args(self, in_maps):
        """Pre-place sharded args on the 8 devices (for re-execution timing)."""
        jax = self.jax
        from jax.sharding import NamedSharding

        sharding = NamedSharding(self.mesh, self.pspec)
        return [jax.device_put(a, sharding) for a in self._concat_args(in_maps)]

    def execute(self, dev_args):
        """Run on pre-placed device args without donation; returns jax arrays."""
        return self._fn(False)(*dev_args)

    def run(self, in_maps):
        out_arrs = self._fn(True)(*self._concat_args(in_maps))
        out_arrs = [np.asarray(a) for a in out_arrs]
        return [
            {
                name: out_arrs[i].reshape(NCORES, *self.out_avals[i].shape)[c]
                for i, name in enumerate(self.out_names)
            }
            for c in range(NCORES)
        ]


def get_runner():
    if "runner" not in _CACHE:
        _CACHE["runner"] = _Runner(_get_program())
    return _CACHE["runner"]


def get_loop_runner(loop_iters, **opts):
    key = ("runner", loop_iters, tuple(sorted(opts.items())))
    if key not in _CACHE:
        _CACHE[key] = _Runner(_get_program(loop_iters, **opts))
    return _CACHE[key]


def measure_hw_ns(in_maps, k_lo=1, k_hi=129, reps=6, **opts):
    """Per-iteration device time via two For_i loop-count variants."""
    import time as _time
    import jax as _jax

    def bench(runner):
        dev = runner.device_args(in_maps)
        for _ in range(2):
            _jax.block_until_ready(runner.execute(dev))
        best = float("inf")
        for _ in range(reps):
            t0 = _time.perf_counter()
            _jax.block_until_ready(runner.execute(dev))
            best = min(best, _time.perf_counter() - t0)
        return best

    t_lo = bench(get_loop_runner(k_lo, **opts))
    t_hi = bench(get_loop_runner(k_hi, **opts))
    return (t_hi - t_lo) / (k_hi - k_lo) * 1e9, t_lo, t_hi


def get_trivial_runner():
    """Near-empty NEFF (one tiny DMA in->out) to measure dispatch overhead."""
    if "trivial" not in _CACHE:
        nc = bacc.Bacc("TRN2", debug=False, num_devices=NCORES)
        with tile.TileContext(nc) as tc:
            with ExitStack() as ctx:
                tin = nc.dram_tensor("tin", [128, 8], F32, kind="ExternalInput").ap()
                tout = nc.dram_tensor(
                    "tout", [128, 8], F32, kind="ExternalOutput"
                ).ap()
                pool = ctx.enter_context(tc.tile_pool(name="tpool", bufs=1))
                tt = pool.tile([128, 8], F32, name="tt")
                nc.sync.dma_start(tt, tin)
                nc.sync.dma_start(tout, tt)
        nc.compile()
        _CACHE["trivial"] = _Runner(nc)
    return _CACHE["trivial"]


def _to2(a):
    """[256, X] -> [128, 2X] with out[p, u*X + j] = a[128u + p, j]."""
    x = np.asarray(a)
    return np.ascontiguousarray(
        x.reshape(2, 128, x.shape[1]).transpose(1, 0, 2).reshape(128, -1)
    )


def _bf(a):
    return np.ascontiguousarray(np.asarray(a, dtype=ml_dtypes.bfloat16))


def _f32(a):
    return np.ascontiguousarray(np.asarray(a, dtype=np.float32))


def kernel(x, x_encoder, Wq, bq, Wk, bk, Wv, bv, h_pos, w_pos, gamma):
    global LAST_EXEC_TIME_NS
    in_maps = make_in_maps(
        x, x_encoder, Wq, bq, Wk, bk, Wv, bv, h_pos, w_pos, gamma
    )
    runner = get_runner()
    results = runner.run(in_maps)

    out = np.empty((B, C, N), np.float32)
    for core in range(NCORES):
        b, half = divmod(core, 2)
        yc = results[core]["y"]  # [128, 2*NH]
        out[b][:, half * NH : (half + 1) * NH] = (
            yc.reshape(128, 2, NH).transpose(1, 0, 2).reshape(C, NH)
        )
    return out.reshape(B, C, H, W)


def make_in_maps(x, x_encoder, Wq, bq, Wk, bk, Wv, bv, h_pos, w_pos, gamma):
    """Host-side input prep shared by kernel() and timing harnesses."""
    x = _f32(x)
    x_encoder = _f32(x_encoder)
    Wq, bq, Wk, bk, Wv, bv = map(_f32, (Wq, bq, Wk, bk, Wv, bv))
    h_pos, w_pos, gamma = map(_f32, (h_pos, w_pos, gamma))
    xf = x.reshape(B, C, N)
    xe = x_encoder.reshape(B, C, N)
    pos = (h_pos + w_pos).reshape(C8, N)
    kb = bk[:, None] + pos  # [32, 4096]
    # strip layout: kbias4[32i+o, j] = kb[o, 1024i + j]
    kbias = _bf(kb.reshape(C8, 4, 8 * 128).transpose(1, 0, 2).reshape(128, 8 * 128))
    wqT = Wq.T  # [256, 32]
    wq_h = _bf(
        np.concatenate(
            [np.tile(wqT[128 * u : 128 * (u + 1)], (1, 4)) for u in range(2)],
            axis=1,
        )
    )  # [128, 256]: wq_h[p, 128u + 32i + o] = Wq[o, 128u + p]
    wk_h = _bf(_to2(Wk.T))
    wv_h = _bf(_to2(Wv.T))
    bq_h = _f32(np.tile(bq, 4)[:, None])  # [128, 1]
    g = float(gamma.reshape(-1)[0])
    consts = np.empty((128, 3), np.float32)
    consts[:, 0] = g
    consts[:, 1] = g * bv[0:128]
    consts[:, 2] = g * bv[128:256]
    in_maps = []
    for core in range(NCORES):
        b, half = divmod(core, 2)
        xq = _to2(xf[b][:, half * NH : (half + 1) * NH])
        in_maps.append(
            {
                "xqf": _f32(xq),
                "xqb": _bf(xq),
                "xeb": _bf(_to2(xe[b])),
                "wq": wq_h,
                "wk": wk_h,
                "wv": wv_h,
                "kbias": kbias,
                "bq": bq_h,
                "consts": consts,
            }
        )
    return in_maps


if __name__ == "__main__":
    import reference

    inputs = {k: np.asarray(v) for k, v in reference.setup_inputs().items()}
    got = kernel(**inputs)
    print("kernel ran; output shape", got.shape, "exec_ns", LAST_EXEC_TIME_NS)



# revision 10
# speedup vs baseline: 1.3257x; 1.1661x over previous
"""Trainium2 Bass kernel for nn_AttentionDecoder (B=4, C=256, H=W=64).

Math (per batch b):
    q  = Wq @ x[b]  + bq          [32, N]   (as qT on device: [32, N] with o on partitions)
    k' = Wk @ xe[b] + bk + pos    [32, N]
    v  = Wv @ xe[b]               [256, N]  (bv folded into epilogue: sum(attn)=1)
    eT = k'^T-chunks: energy^T[m, n] = sum_o k'[o,m] q[o,n]
    pT = exp(eT)                  (no max-subtraction: |energy| < ~30, fp32-exp safe)
    out[c, n] = sum_m v[c, m] pT[m, n]      (PE: lhsT=vT chunk, rhs=pT chunk)
    s[n]      = sum_m pT[m, n]              (PE: lhsT=ones -> replicated rows)
    y = gamma * (out / s + bv) + x

Sharding: 8 cores = (batch, query-half). Each core: 2048 query rows, full m=4096.
"""

import numpy as np
import ml_dtypes
from contextlib import ExitStack

import concourse.bass as bass
import concourse.bass_isa as bass_isa
import concourse.bacc as bacc
import concourse.tile as tile
import concourse.mybir as mybir
from concourse.bass import ds, ts

B, C, H, W = 4, 256, 64, 64
N = H * W          # 4096
C8 = 32
NH = N // 2        # 2048 query rows per core
NCORES = 8
NG = NH // 512     # 4 n-groups of 512 per core
F32 = mybir.dt.float32
BF16 = mybir.dt.bfloat16
AF = mybir.ActivationFunctionType
OP = mybir.AluOpType

LAST_EXEC_TIME_NS = None
_CACHE = {}


def build_attention_v2(
    ctx,
    tc,
    y,
    ins,
    lag=2,
    vt_act_share=2,
    finish_at=0,
    st_a_pool=False,
    s_allreduce="pool",
    pool_dma=False,
):
    """v2: stall-free PE schedule.

    - energy psum double-buffered as [128,2,512] pairs (4 banks), exp in
      2-slot instructions so round J+1's energy never waits a 4-slot drain
    - pv psum [128,2,512] double-buffered across groups (4 banks); the
      epilogue STTs read pv psum directly (no o0/o1 evacuation)
    - s-reduction: slot-tree split across Pool (st_a) and DVE, reduced to
      [128,512] with only the last-8-slot fold on the tail critical path;
      final cross-partition sum via gpsimd partition_all_reduce (no PE
      s-matmuls, no s psum bank)
    - input DMA split across SP and Pool queues in consumption order;
      vT psum evacuation split DVE/ACT; q bias-add on ACT (Copy+bias)
    """
    nc = tc.nc
    y_v = y.rearrange("p (u n) -> p u n", u=2)
    xqf_d = ins["xqf"].rearrange("p (u n) -> p u n", u=2)
    xqb_d = ins["xqb"].rearrange("p (u n) -> p u n", u=2)
    xeb_d = ins["xeb"].rearrange("p (u m) -> p u m", u=2)

    singles = ctx.enter_context(tc.tile_pool(name="singles", bufs=1))

    # ---- resident SBUF tensors ----
    wq_sb = singles.tile([128, 2, 128], BF16, name="wq_sb")
    bq_sb = singles.tile([128, 1], F32, name="bq_sb")
    wk_sb = singles.tile([128, 2 * C8], BF16, name="wk_sb")
    kbias_sb = singles.tile([128, 8 * 128], BF16, name="kbias_sb")
    wv_sb = singles.tile([128, 2 * C], BF16, name="wv_sb")
    consts_sb = singles.tile([128, 3], F32, name="consts_sb")
    xqb_sb = singles.tile([128, 2, NH], BF16, name="xqb_sb")
    xeb_sb = singles.tile([128, 2, N], BF16, name="xeb_sb")
    xqf_sb = singles.tile([128, 2, NH], F32, name="xqf_sb")
    qT4_sb = singles.tile([128, NH], BF16, name="qT4_sb")
    k4_sb = singles.tile([128, 8 * 128], BF16, name="k4_sb")
    vT_sb = singles.tile([128, 32 * C], BF16, name="vT_sb")

    # SP DMA queue, ordered to match PE consumption: q prod, k prod, vT prod
    def xqb_dma(dma, g4):
        for u in range(2):
            dma(xqb_sb[:, u, ds(512 * g4, 512)], xqb_d[:, u, ds(512 * g4, 512)])

    def xeb_dma(dma, quarter):
        for u in range(2):
            dma(
                xeb_sb[:, u, ds(1024 * quarter, 1024)],
                xeb_d[:, u, ds(1024 * quarter, 1024)],
            )

    dma2 = nc.gpsimd.dma_start if pool_dma else nc.sync.dma_start
    nc.sync.dma_start(wq_sb, ins["wq"].rearrange("p (u j) -> p u j", u=2))
    nc.sync.dma_start(bq_sb, ins["bq"])
    xqb_dma(nc.sync.dma_start, 0)
    nc.sync.dma_start(wk_sb, ins["wk"])
    if pool_dma:
        xeb_dma(nc.sync.dma_start, 0)
        nc.sync.dma_start(kbias_sb, ins["kbias"])
        for quarter in range(1, 4):
            xeb_dma(nc.sync.dma_start, quarter)
        dma2(wv_sb, ins["wv"])
        dma2(consts_sb, ins["consts"])
        for g4 in range(1, NG):
            xqb_dma(dma2, g4)
        dma2(xqf_sb, xqf_d)
    else:
        # single SP queue: interleave by consumption order
        xeb_dma(dma2, 0)
        xqb_dma(dma2, 1)
        dma2(kbias_sb, ins["kbias"])
        dma2(wv_sb, ins["wv"])
        xeb_dma(dma2, 1)
        xqb_dma(dma2, 2)
        xeb_dma(dma2, 2)
        xqb_dma(dma2, 3)
        xeb_dma(dma2, 3)
        dma2(consts_sb, ins["consts"])
        dma2(xqf_sb, xqf_d)

    # warm the Exp ACT table during the production phase (table load ~2.7us)
    act_warm = singles.tile([1, 1], F32, name="act_warm")
    nc.scalar.activation(out=act_warm, in_=bq_sb[0:1, :], func=AF.Exp)

    # ---- productions ----
    vT32 = vT_sb.rearrange("p (d c) -> p d c", c=C)
    with tc.tile_pool(name="psqp", bufs=2, space="PSUM") as psqp, tc.tile_pool(
        name="pskp", bufs=1, space="PSUM"
    ) as pskp, tc.tile_pool(name="psvp", bufs=3, space="PSUM") as psvp:

        def emit_q(g4):
            psq = psqp.tile([128, 512], F32, name="psq")
            for u in range(2):
                nc.tensor.matmul(
                    psq,
                    lhsT=wq_sb[:, u, :],
                    rhs=xqb_sb[:, u, ds(512 * g4, 512)],
                    start=(u == 0),
                    stop=(u == 1),
                )
            nc.scalar.activation(
                out=qT4_sb[:, ds(512 * g4, 512)], in_=psq, func=AF.Identity, bias=bq_sb
            )

        psk4 = pskp.tile([128, 8 * 128], F32, name="psk4")

        def emit_k(c8):
            i, j = c8 // 2, c8 % 2
            for u in range(2):
                nc.tensor.matmul(
                    psk4[ds(C8 * i, C8), ds(512 * j, 512)],
                    lhsT=wk_sb[:, ds(C8 * u, C8)],
                    rhs=xeb_sb[:, u, ds(512 * c8, 512)],
                    start=(u == 0),
                    stop=(u == 1),
                    tile_position=(0, C8 * i),
                )

        def emit_vt(t):
            # pair of m-positions (2t, 2t+1); evac alternates DVE/ACT
            psv = psvp.tile([128, 2, C], F32, name="psv")
            for w in range(2):
                for u in range(2):
                    nc.tensor.matmul(
                        psv[:, w, :],
                        lhsT=xeb_sb[:, u, ds(128 * (2 * t + w), 128)],
                        rhs=wv_sb[:, ds(C * u, C)],
                        start=(u == 0),
                        stop=(u == 1),
                    )
            dest = vT32[:, ds(2 * t, 2), :]
            if vt_act_share and t % vt_act_share == 0:
                nc.scalar.copy(dest, psv)
            else:
                nc.vector.tensor_copy(dest, psv)

        emit_q(0)
        emit_k(0)
        emit_k(1)
        emit_q(1)
        emit_vt(0)
        emit_vt(1)
        emit_k(2)
        emit_k(3)
        emit_q(2)
        emit_vt(2)
        emit_vt(3)
        emit_vt(4)
        emit_vt(5)
        emit_k(4)
        emit_k(5)
        emit_q(3)
        emit_vt(6)
        emit_vt(7)
        emit_k(6)
        emit_k(7)
        nc.vector.tensor_add(k4_sb, psk4, kbias_sb)
        for t in range(8, 16):
            emit_vt(t)

    # ---- main loop ----
    ppool = ctx.enter_context(tc.tile_pool(name="ppool", bufs=2))
    epool = ctx.enter_context(tc.tile_pool(name="epool", bufs=2, space="PSUM"))
    mpsum = ctx.enter_context(tc.tile_pool(name="mpsum", bufs=2, space="PSUM"))
    work = ctx.enter_context(tc.tile_pool(name="work", bufs=2))
    # PSUM budget: epool 2 tiles x 2 banks + mpsum 2 tiles x 2 banks = 8

    def slot_to_chunk(s):
        return 8 * (s % 4) + s // 4

    LAG = lag

    def finish(p):
        """Tail-8-slot fold + s + normalize + residual + store for group p
        (runs during the next group's early rounds; reads pv psum)."""
        gp = p["g"]
        b2 = work.tile([128, 4, 512], BF16, name="b2")
        nc.vector.tensor_add(b2, p["pT"][:, 24:28, :], p["pT"][:, 28:32, :])
        b2a = work.tile([128, 2, 512], BF16, name="b2a")
        nc.vector.tensor_add(b2a, b2[:, 0:2, :], b2[:, 2:4, :])
        b2b = work.tile([128, 512], BF16, name="b2b")
        nc.vector.tensor_add(b2b, b2a[:, 0, :], b2a[:, 1, :])
        st = work.tile([128, 512], BF16, name="st")
        nc.vector.tensor_add(st, p["c3"], b2b)
        s_rep = work.tile([128, 512], F32, name="s_rep")
        if s_allreduce == "pool":
            nc.gpsimd.partition_all_reduce(
                s_rep, st, channels=128, reduce_op=bass_isa.ReduceOp.add
            )
        else:  # timing-only stub (wrong s, gamma=0 output unaffected)
            nc.vector.memset(s_rep, 1.0)
        r_rep = work.tile([128, 512], F32, name="r_rep")
        nc.vector.reciprocal(r_rep, s_rep)
        for u in range(2):
            t = work.tile([128, 512], F32, name="t")
            nc.vector.scalar_tensor_tensor(
                out=t,
                in0=p["pv"][:, u, :],
                scalar=consts_sb[:, 0:1],
                in1=r_rep,
                op0=OP.mult,
                op1=OP.mult,
            )
            yt = work.tile([128, 512], F32, name="yt")
            nc.vector.scalar_tensor_tensor(
                out=yt,
                in0=t,
                scalar=consts_sb[:, u + 1 : u + 2],
                in1=xqf_sb[:, u, ds(512 * gp, 512)],
                op0=OP.add,
                op1=OP.add,
            )
            nc.sync.dma_start(y_v[:, u, ds(512 * gp, 512)], yt)

    pending = None
    for g in range(NG):
        pT = ppool.tile([128, 32, 512], BF16, name="pT")
        pv = mpsum.tile([128, 2, 512], F32, name="pv")
        cur = {"g": g, "pT": pT, "pv": pv}
        for J in range(8 + LAG):
            if J < 8:
                for h in ((1, 0) if J == 7 else (0, 1)):
                    e2 = epool.tile([128, 2, 512], F32, name="e2")
                    for k in range(2):
                        i = 2 * h + k
                        nc.tensor.matmul(
                            e2[:, k, :],
                            lhsT=k4_sb[ds(C8 * i, C8), ds(128 * J, 128)],
                            rhs=qT4_sb[ds(C8 * i, C8), ds(512 * g, 512)],
                            start=True,
                            stop=True,
                            tile_position=(C8 * i, 0),
                        )
                    nc.scalar.activation(
                        out=pT[:, ds(4 * J + 2 * h, 2), :], in_=e2, func=AF.Exp
                    )
            if J >= LAG and J - LAG < 8:
                for j in range(4):
                    s = 4 * (J - LAG) + j
                    mc = slot_to_chunk(s)
                    st = s == 0
                    sp = s == 31
                    for u in range(2):
                        nc.tensor.matmul(
                            pv[:, u, :],
                            lhsT=vT_sb[:, ds(C * mc + 128 * u, 128)],
                            rhs=pT[:, s, :],
                            start=st,
                            stop=sp,
                        )
            if J == 4:
                # Pool is otherwise idle: big first-level fold of slots 0:16
                cur["st_a"] = work.tile([128, 8, 512], BF16, name="st_a")
                (nc.gpsimd if st_a_pool else nc.vector).tensor_add(
                    cur["st_a"], pT[:, 0:8, :], pT[:, 8:16, :]
                )
            if J == 5:
                cur["b1"] = work.tile([128, 4, 512], BF16, name="b1")
                nc.vector.tensor_add(cur["b1"], pT[:, 16:20, :], pT[:, 20:24, :])
            if J == 6:
                cur["a2"] = work.tile([128, 4, 512], BF16, name="a2")
                nc.vector.tensor_add(
                    cur["a2"], cur["st_a"][:, 0:4, :], cur["st_a"][:, 4:8, :]
                )
                cur["cc"] = work.tile([128, 4, 512], BF16, name="cc")
                nc.vector.tensor_add(cur["cc"], cur["a2"], cur["b1"])
            if J == 7:
                cur["c2"] = work.tile([128, 2, 512], BF16, name="c2")
                nc.vector.tensor_add(
                    cur["c2"], cur["cc"][:, 0:2, :], cur["cc"][:, 2:4, :]
                )
                cur["c3"] = work.tile([128, 512], BF16, name="c3")
                nc.vector.tensor_add(cur["c3"], cur["c2"][:, 0, :], cur["c2"][:, 1, :])
            if J == finish_at and pending is not None:
                finish(pending)
                pending = None
        pending = cur
    finish(pending)


def build_attention(
    ctx,
    tc,
    y,
    ins,
    energy_mode="pack4",
    skip_s=False,
    lag=2,
    tree_levels=3,
    eps_split=False,
    wbufs=2,
    skip_recip=False,
    pv_first=False,
    finish_at=2,
):
    """y: [128, 2*NH] f32 dram AP.  ins: dict of dram APs (see kernel()).

    energy_mode:
      'pack4'  - 4x row-tiled K=32 matmuls (tile_position), k' strip-split
      'k32'    - plain K=32 matmuls at partitions 0:32 (no tile_position)
      'rep128' - k' replicated on all 4 strips, full K=128 matmul computes
                 4x energy; the 1/4 is folded into exp's scale (free affine)
    """
    nc = tc.nc
    y_v = y.rearrange("p (u n) -> p u n", u=2)
    xqf_d = ins["xqf"].rearrange("p (u n) -> p u n", u=2)
    xqb_d = ins["xqb"].rearrange("p (u n) -> p u n", u=2)
    xeb_d = ins["xeb"].rearrange("p (u m) -> p u m", u=2)

    singles = ctx.enter_context(tc.tile_pool(name="singles", bufs=1))

    # ---- resident SBUF tensors; DMA order matters: what productions need
    # first goes first, the residual input (only needed ~60us in) goes last.
    wq_sb = singles.tile([128, 2, 128], BF16, name="wq_sb")
    nc.sync.dma_start(wq_sb, ins["wq"].rearrange("p (u j) -> p u j", u=2))
    wk_sb = singles.tile([128, 2 * C8], BF16, name="wk_sb")
    nc.sync.dma_start(wk_sb, ins["wk"])
    wv_sb = singles.tile([128, 2 * C], BF16, name="wv_sb")
    nc.sync.dma_start(wv_sb, ins["wv"])
    bq_sb = singles.tile([128, 1], F32, name="bq_sb")
    nc.sync.dma_start(bq_sb, ins["bq"])
    kbias_sb = singles.tile([128, 8 * 128], BF16, name="kbias_sb")
    nc.sync.dma_start(kbias_sb, ins["kbias"])
    consts_sb = singles.tile([128, 3], F32, name="consts_sb")
    nc.sync.dma_start(consts_sb, ins["consts"])
    # xqb split per (c-chunk, n-group) so qT production starts on first chunk
    xqb_sb = singles.tile([128, 2, NH], BF16, name="xqb_sb")
    for g4 in range(NG):
        for u in range(2):
            nc.sync.dma_start(
                xqb_sb[:, u, ds(512 * g4, 512)], xqb_d[:, u, ds(512 * g4, 512)]
            )
    # xeb split into chunks so k/v production starts before the full 2MB lands
    xeb_sb = singles.tile([128, 2, N], BF16, name="xeb_sb")
    for u in range(2):
        for quarter in range(4):
            nc.sync.dma_start(
                xeb_sb[:, u, ds(1024 * quarter, 1024)],
                xeb_d[:, u, ds(1024 * quarter, 1024)],
            )
    xqf_sb = singles.tile([128, 2, NH], F32, name="xqf_sb")
    nc.sync.dma_start(xqf_sb, xqf_d)

    ones_sb = singles.tile([128, 128], BF16, name="ones_sb")
    nc.vector.memset(ones_sb, 1.0)

    # warm the Exp ACT table during the production phase (table load ~2.7us)
    act_warm = singles.tile([1, 1], F32, name="act_warm")
    nc.scalar.activation(out=act_warm, in_=bq_sb[0:1, :], func=AF.Exp)

    # qT4: qT replicated at 4 partition strips (strip i holds qT[o, :] at
    # partitions 32i+o) for the row-packed / replicated energyT matmuls.
    qT4_sb = singles.tile([128, NH], BF16, name="qT4_sb")
    # k4: k' distributed over strips: strip i = m in [1024i, 1024(i+1))
    assert energy_mode == "pack4", "col-tiled k production supports pack4 only"
    k4_sb = singles.tile([128, 8 * 128], BF16, name="k4_sb")
    vT_sb = singles.tile([128, 32 * C], BF16, name="vT_sb")

    # ---- productions ----
    with tc.tile_pool(name="prodpsum", bufs=2, space="PSUM") as prodpsum:
        # qT4[32i+o, n] = sum_c Wq[o, c] x[c, n] + bq[o]   (wq host-tiled x4)
        for g4 in range(NG):
            psq = prodpsum.tile([128, 512], F32, name="psq")
            for u in range(2):
                nc.tensor.matmul(
                    psq,
                    lhsT=wq_sb[:, u, :],
                    rhs=xqb_sb[:, u, ds(512 * g4, 512)],
                    start=(u == 0),
                    stop=(u == 1),
                )
            nc.vector.tensor_scalar_add(qT4_sb[:, ds(512 * g4, 512)], psq, bq_sb)
        # k'[o, m] = sum_c Wk[o, c] xe[c, m] + kbias[o, m], produced
        # col-tiled (4 concurrent partition strips) directly in strip layout
        psk4 = prodpsum.tile([128, 8 * 128], F32, name="psk4", bufs=1)
        for c8 in range(N // 512):
            i, j = c8 // 2, c8 % 2
            for u in range(2):
                nc.tensor.matmul(
                    psk4[ds(C8 * i, C8), ds(512 * j, 512)],
                    lhsT=wk_sb[:, ds(C8 * u, C8)],
                    rhs=xeb_sb[:, u, ds(512 * c8, 512)],
                    start=(u == 0),
                    stop=(u == 1),
                    tile_position=(0, C8 * i),
                )
        nc.vector.tensor_add(k4_sb, psk4, kbias_sb)
        # vT[m-chunk mc][mm, co] = sum_c xe[c, 128*mc+mm] Wv[co, c]
        def emit_vt_chunk(pool, mc):
            psv = pool.tile([128, C], F32, name="psv")
            for u in range(2):
                nc.tensor.matmul(
                    psv,
                    lhsT=xeb_sb[:, u, ds(128 * mc, 128)],
                    rhs=wv_sb[:, ds(C * u, C)],
                    start=(u == 0),
                    stop=(u == 1),
                )
            nc.vector.tensor_copy(vT_sb[:, ds(C * mc, C)], psv)

        for mc in range(32):
            emit_vt_chunk(prodpsum, mc)

    # ---- main loop: intra-group pipeline with 2-J-step lag.
    # energyT round J: 4 row-packed K=32 matmuls (strips i=0..3) -> 4 psum
    # banks; exp writes pT slots 4J..4J+3. PV of slots 4(J-2).. runs two
    # J-steps behind so PE has work while ACT drains exp. The s-reduction
    # (DVE tree 32->8 chunks + 8 partition-sum matmuls) and the epilogue of
    # group g overlap group g+1's pipeline fill.
    ppool = ctx.enter_context(tc.tile_pool(name="ppool", bufs=2))
    epool = ctx.enter_context(tc.tile_pool(name="epool", bufs=1, space="PSUM"))
    work = ctx.enter_context(tc.tile_pool(name="work", bufs=wbufs))
    mpsum = ctx.enter_context(tc.tile_pool(name="mpsum", bufs=1, space="PSUM"))
    # PSUM budget: epool 4 + pv0/pv1/s_ps 3 = 7 of 8 banks

    def slot_to_chunk(s):
        return 8 * (s % 4) + s // 4 if energy_mode == "pack4" else s

    LAG = lag

    def finish(p):
        """Tree-tail + s-matmuls + normalize + residual + store for group p."""
        gp = p["g"]
        s_ps = mpsum.tile([128, 512], F32, name="s_ps")
        if skip_s:
            nc.vector.memset(s_ps, 1.0)
        else:
            st_b = work.tile([128, 8, 512], BF16, name="st_b", bufs=1)
            nc.vector.tensor_add(
                st_b, p["pT"][:, 16:24, :], p["pT"][:, 24:32, :]
            )
            st_c = work.tile([128, 8, 512], BF16, name="st_c", bufs=1)
            nc.vector.tensor_add(st_c, p["st_a"], st_b)
            st3 = work.tile([128, 4, 512], BF16, name="st3", bufs=1)
            nc.vector.tensor_add(st3, st_c[:, 0:4, :], st_c[:, 4:8, :])
            for s8 in range(4):
                nc.tensor.matmul(
                    s_ps,
                    lhsT=ones_sb,
                    rhs=st3[:, s8, :],
                    start=(s8 == 0),
                    stop=(s8 == 3),
                )
        r_rep = work.tile([128, 512], F32, name="r_rep")
        if skip_recip:
            nc.vector.memset(r_rep, 1.0)
        else:
            nc.vector.reciprocal(r_rep, s_ps)
        for u, ou in enumerate([p["o0"], p["o1"]]):
            t = work.tile([128, 512], F32, name="t")
            nc.vector.scalar_tensor_tensor(
                out=t,
                in0=ou,
                scalar=consts_sb[:, 0:1],
                in1=r_rep,
                op0=OP.mult,
                op1=OP.mult,
            )
            yt = work.tile([128, 512], F32, name="yt")
            nc.vector.scalar_tensor_tensor(
                out=yt,
                in0=t,
                scalar=consts_sb[:, u + 1 : u + 2],
                in1=xqf_sb[:, u, ds(512 * gp, 512)],
                op0=OP.add,
                op1=OP.add,
            )
            nc.sync.dma_start(y_v[:, u, ds(512 * gp, 512)], yt)

    pending = None
    for g in range(NG):
        pT = ppool.tile([128, 32, 512], BF16, name="pT")
        pv0 = mpsum.tile([128, 512], F32, name="pv0")
        pv1 = mpsum.tile([128, 512], F32, name="pv1")
        st_a = None
        def emit_e(J):
                if eps_split:
                    e_lo = epool.tile([128, 2, 512], F32, name="e_lo")
                    e_hi = epool.tile([128, 2, 512], F32, name="e_hi")
                    halves = [e_lo, e_hi]
                    e_ps = None
                else:
                    e_ps = epool.tile([128, 4, 512], F32, name="e_ps")
                for i in range(4):
                    e_out = (
                        halves[i // 2][:, i % 2, :] if eps_split else e_ps[:, i, :]
                    )
                    nc.tensor.matmul(
                        e_out,
                        lhsT=k4_sb[ds(C8 * i, C8), ds(128 * J, 128)],
                        rhs=qT4_sb[ds(C8 * i, C8), ds(512 * g, 512)],
                        start=True,
                        stop=True,
                        tile_position=(C8 * i, 0),
                    )
                escale = 0.25 if energy_mode == "rep128" else 1.0
                if eps_split:
                    for h in range(2):
                        nc.scalar.activation(
                            out=pT[:, ds(4 * J + 2 * h, 2), :],
                            in_=halves[h],
                            func=AF.Exp,
                            scale=escale,
                        )
                else:
                    nc.scalar.activation(
                        out=pT[:, ds(4 * J, 4), :],
                        in_=e_ps,
                        func=AF.Exp,
                        scale=escale,
                    )

        def emit_pv(J):
                for j in range(4):
                    s = 4 * (J - LAG) + j
                    mc = slot_to_chunk(s)
                    st = s == 0
                    sp = s == 31
                    nc.tensor.matmul(
                        pv0,
                        lhsT=vT_sb[:, ds(C * mc, 128)],
                        rhs=pT[:, s, :],
                        start=st,
                        stop=sp,
                    )
                    nc.tensor.matmul(
                        pv1,
                        lhsT=vT_sb[:, ds(C * mc + 128, 128)],
                        rhs=pT[:, s, :],
                        start=st,
                        stop=sp,
                    )

        for J in range(8 + LAG):
            if pv_first:
                if J >= LAG:
                    emit_pv(J)
                if J < 8:
                    emit_e(J)
            else:
                if J < 8:
                    emit_e(J)
                if J >= LAG:
                    emit_pv(J)
            if J == 4 and not skip_s:
                # first half of the s slot-tree: slots 0..15 are ready
                st_a = work.tile([128, 8, 512], BF16, name="st_a")
                nc.vector.tensor_add(st_a, pT[:, 0:8, :], pT[:, 8:16, :])
            if J == finish_at and pending is not None:
                finish(pending)
                pending = None
        # evacuate PV psum to SBUF right away so the psum banks free for the
        # next group's PV; the tree-tail + s-matmuls + normalize/epilogue are
        # deferred into the next group's J-loop (see finish())
        o0 = work.tile([128, 512], F32, name="o0")
        nc.vector.tensor_copy(o0, pv0)
        o1 = work.tile([128, 512], F32, name="o1")
        nc.vector.tensor_copy(o1, pv1)
        pending = dict(g=g, pT=pT, o0=o0, o1=o1, st_a=st_a)
    finish(pending)


INPUT_SPECS = [
    ("xqf", [128, 2 * NH], F32),
    ("xqb", [128, 2 * NH], BF16),
    ("xeb", [128, 2 * N], BF16),
    ("wq", [128, 2 * 128], BF16),
    ("wk", [128, 2 * C8], BF16),
    ("wv", [128, 2 * C], BF16),
    ("kbias", [128, 8 * 128], BF16),
    ("bq", [128, 1], F32),
    ("consts", [128, 3], F32),
]


def _get_program(loop_iters=None, version=2, **opts):
    """loop_iters=None: plain program. loop_iters=k: whole kernel wrapped in a
    device-side For_i loop (for HW timing: slope between two loop counts)."""
    key = ("nc", loop_iters, version, tuple(sorted(opts.items())))
    if key not in _CACHE:
        build = build_attention_v2 if version == 2 else build_attention
        nc = bacc.Bacc("TRN2", debug=False, num_devices=NCORES)
        with tile.TileContext(nc) as tc:
            with ExitStack() as ctx:
                ins = {
                    name: nc.dram_tensor(name, shape, dt, kind="ExternalInput").ap()
                    for name, shape, dt in INPUT_SPECS
                }
                y = nc.dram_tensor("y", [128, 2 * NH], F32, kind="ExternalOutput").ap()
                if loop_iters is None:
                    build(ctx, tc, y, ins, **opts)
                else:
                    # hint_engines: body >256 insts/engine, so the back-edge
                    # would otherwise pay an IRAM refetch (~3-4us) per iter
                    with tc.For_i(
                        0,
                        loop_iters,
                        1,
                        hint_engines=(
                            mybir.EngineType.PE,
                            mybir.EngineType.Activation,
                            mybir.EngineType.DVE,
                            mybir.EngineType.Pool,
                        ),
                    ):
                        with ExitStack() as inner:
                            build(inner, tc, y, ins, **opts)
        nc.compile()
        _CACHE[key] = nc
    return _CACHE[key]


class _Runner:
    """Executes the compiled Bass program on 8 cores via PJRT/axon.

    Mirrors bass2jax.run_bass_via_pjrt's multi-core path, but keeps the
    jitted callable so repeated executions don't re-lower, and supports
    chaining `iters` NEFF executions inside one program (each iteration's
    outputs feed the next iteration's output buffers, creating a data
    dependency) so per-execution device time can be measured without
    host dispatch overhead.
    """

    def __init__(self, nc):
        import jax
        from jax.experimental.shard_map import shard_map
        from jax.sharding import Mesh, PartitionSpec
        from concourse import bass2jax

        bass2jax.install_neuronx_cc_hook()
        self.nc = nc
        self.jax = jax
        in_names, out_names, out_avals, zero_outs = [], [], [], []
        partition_name = (
            nc.partition_id_tensor.name if nc.partition_id_tensor else None
        )
        for alloc in nc.m.functions[0].allocations:
            if not isinstance(alloc, mybir.MemoryLocationSet):
                continue
            name = alloc.memorylocations[0].name
            if alloc.kind == "ExternalInput":
                if name != partition_name:
                    in_names.append(name)
            elif alloc.kind == "ExternalOutput":
                out_names.append(name)
                shape = tuple(alloc.tensor_shape)
                dtype = mybir.dt.np(alloc.dtype)
                out_avals.append(jax.core.ShapedArray(shape, dtype))
                zero_outs.append(np.zeros(shape, dtype))
        self.n_params = len(in_names)
        self.n_outs = len(out_avals)
        self.out_names = out_names
        self.out_avals = out_avals
        self.zero_outs = zero_outs
        all_in_names = list(in_names) + list(out_names)
        if partition_name is not None:
            all_in_names.append(partition_name)
        self.in_names = in_names
        self.partition_name = partition_name

        devices = jax.devices()[:NCORES]
        assert len(devices) == NCORES
        mesh = Mesh(np.asarray(devices), ("core",))
        donate = tuple(range(self.n_params, self.n_params + self.n_outs))
        out_avals_t = tuple(out_avals)
        all_in_names_t = tuple(all_in_names)
        out_names_t = tuple(out_names)

        self.mesh = mesh
        self.pspec = PartitionSpec("core")

        def make(donated):
            def _body(*args):
                operands = list(args)
                if partition_name is not None:
                    operands.append(bass2jax.partition_id_tensor())
                outs = bass2jax._bass_exec_p.bind(
                    *operands,
                    out_avals=out_avals_t,
                    in_names=all_in_names_t,
                    out_names=out_names_t,
                    lowering_input_output_aliases=(),
                    sim_require_finite=True,
                    sim_require_nnan=True,
                    nc=nc,
                )
                return tuple(outs)

            in_specs = (PartitionSpec("core"),) * (self.n_params + self.n_outs)
            out_specs = (PartitionSpec("core"),) * self.n_outs
            return jax.jit(
                shard_map(
                    _body,
                    mesh=mesh,
                    in_specs=in_specs,
                    out_specs=out_specs,
                    check_rep=False,
                ),
                donate_argnums=donate if donated else (),
                keep_unused=True,
            )

        self._make = make
        self._fns = {}

    def _fn(self, donated):
        if donated not in self._fns:
            self._fns[donated] = self._make(donated)
        return self._fns[donated]

    def _concat_args(self, in_maps):
        concat_in = [
            np.concatenate([np.asarray(m[name]) for m in in_maps], axis=0)
            for name in self.in_names
        ]
        concat_zeros = [
            np.zeros((NCORES * z.shape[0], *z.shape[1:]), z.dtype)
            for z in self.zero_outs
        ]
        return concat_in + concat_zeros

    def device_args(self, in_maps):
        """Pre-place sharded args on the 8 devices (for re-execution timing)."""
        jax = self.jax
        from jax.sharding import NamedSharding

        sharding = NamedSharding(self.mesh, self.pspec)
        return [jax.device_put(a, sharding) for a in self._concat_args(in_maps)]

    def execute(self, dev_args):
        """Run on pre-placed device args without donation; returns jax arrays."""
        return self._fn(False)(*dev_args)

    def run(self, in_maps):
        out_arrs = self._fn(True)(*self._concat_args(in_maps))
        out_arrs = [np.asarray(a) for a in out_arrs]
        return [
            {
                name: out_arrs[i].reshape(NCORES, *self.out_avals[i].shape)[c]
                for i, name in enumerate(self.out_names)
            }
            for c in range(NCORES)
        ]


def get_runner():
    if "runner" not in _CACHE:
        _CACHE["runner"] = _Runner(_get_program())
    return _CACHE["runner"]


def get_loop_runner(loop_iters, **opts):
    key = ("runner", loop_iters, tuple(sorted(opts.items())))
    if key not in _CACHE:
        _CACHE[key] = _Runner(_get_program(loop_iters, **opts))
    return _CACHE[key]


def measure_hw_ns(in_maps, k_lo=1, k_hi=129, reps=6, **opts):
    """Per-iteration device time via two For_i loop-count variants."""
    import time as _time
    import jax as _jax

    def bench(runner):
        dev = runner.device_args(in_maps)
        for _ in range(2):
            _jax.block_until_ready(runner.execute(dev))
        best = float("inf")
        for _ in range(reps):
            t0 = _time.perf_counter()
            _jax.block_until_ready(runner.execute(dev))
            best = min(best, _time.perf_counter() - t0)
        return best

    t_lo = bench(get_loop_runner(k_lo, **opts))
    t_hi = bench(get_loop_runner(k_hi, **opts))
    return (t_hi - t_lo) / (k_hi - k_lo) * 1e9, t_lo, t_hi


def get_trivial_runner():
    """Near-empty NEFF (one tiny DMA in->out) to measure dispatch overhead."""
    if "trivial" not in _CACHE:
        nc = bacc.Bacc("TRN2", debug=False, num_devices=NCORES)
        with tile.TileContext(nc) as tc:
            with ExitStack() as ctx:
                tin = nc.dram_tensor("tin", [128, 8], F32, kind="ExternalInput").ap()
                tout = nc.dram_tensor(
                    "tout", [128, 8], F32, kind="ExternalOutput"
                ).ap()
                pool = ctx.enter_context(tc.tile_pool(name="tpool", bufs=1))
                tt = pool.tile([128, 8], F32, name="tt")
                nc.sync.dma_start(tt, tin)
                nc.sync.dma_start(tout, tt)
        nc.compile()
        _CACHE["trivial"] = _Runner(nc)
    return _CACHE["trivial"]


def _to2(a):
    """[256, X] -> [128, 2X] with out[p, u*X + j] = a[128u + p, j]."""
    x = np.asarray(a)
    return np.ascontiguousarray(
        x.reshape(2, 128, x.shape[1]).transpose(1, 0, 2).reshape(128, -1)
    )


def _bf(a):
    return np.ascontiguousarray(np.asarray(a, dtype=ml_dtypes.bfloat16))


def _f32(a):
    return np.ascontiguousarray(np.asarray(a, dtype=np.float32))


def kernel(x, x_encoder, Wq, bq, Wk, bk, Wv, bv, h_pos, w_pos, gamma):
    global LAST_EXEC_TIME_NS
    in_maps = make_in_maps(
        x, x_encoder, Wq, bq, Wk, bk, Wv, bv, h_pos, w_pos, gamma
    )
    runner = get_runner()
    results = runner.run(in_maps)

    out = np.empty((B, C, N), np.float32)
    for core in range(NCORES):
        b, half = divmod(core, 2)
        yc = results[core]["y"]  # [128, 2*NH]
        out[b][:, half * NH : (half + 1) * NH] = (
            yc.reshape(128, 2, NH).transpose(1, 0, 2).reshape(C, NH)
        )
    return out.reshape(B, C, H, W)


def make_in_maps(x, x_encoder, Wq, bq, Wk, bk, Wv, bv, h_pos, w_pos, gamma):
    """Host-side input prep shared by kernel() and timing harnesses."""
    x = _f32(x)
    x_encoder = _f32(x_encoder)
    Wq, bq, Wk, bk, Wv, bv = map(_f32, (Wq, bq, Wk, bk, Wv, bv))
    h_pos, w_pos, gamma = map(_f32, (h_pos, w_pos, gamma))
    xf = x.reshape(B, C, N)
    xe = x_encoder.reshape(B, C, N)
    pos = (h_pos + w_pos).reshape(C8, N)
    kb = bk[:, None] + pos  # [32, 4096]
    # strip layout: kbias4[32i+o, j] = kb[o, 1024i + j]
    kbias = _bf(kb.reshape(C8, 4, 8 * 128).transpose(1, 0, 2).reshape(128, 8 * 128))
    wqT = Wq.T  # [256, 32]
    wq_h = _bf(
        np.concatenate(
            [np.tile(wqT[128 * u : 128 * (u + 1)], (1, 4)) for u in range(2)],
            axis=1,
        )
    )  # [128, 256]: wq_h[p, 128u + 32i + o] = Wq[o, 128u + p]
    wk_h = _bf(_to2(Wk.T))
    wv_h = _bf(_to2(Wv.T))
    bq_h = _f32(np.tile(bq, 4)[:, None])  # [128, 1]
    g = float(gamma.reshape(-1)[0])
    consts = np.empty((128, 3), np.float32)
    consts[:, 0] = g
    consts[:, 1] = g * bv[0:128]
    consts[:, 2] = g * bv[128:256]
    in_maps = []
    for core in range(NCORES):
        b, half = divmod(core, 2)
        xq = _to2(xf[b][:, half * NH : (half + 1) * NH])
        in_maps.append(
            {
                "xqf": _f32(xq),
                "xqb": _bf(xq),
                "xeb": _bf(_to2(xe[b])),
                "wq": wq_h,
                "wk": wk_h,
                "wv": wv_h,
                "kbias": kbias,
                "bq": bq_h,
                "consts": consts,
            }
        )
    return in_maps


if __name__ == "__main__":
    import reference

    inputs = {k: np.asarray(v) for k, v in reference.setup_inputs().items()}
    got = kernel(**inputs)
    print("kernel ran; output shape", got.shape, "exec_ns", LAST_EXEC_TIME_NS)

